# revision 1
# baseline (speedup 1.0000x reference)
"""Trainium2 Bass kernel for nn_ConvAttLIF: Conv2d(64->128, 3x3, pad1) over
(B=8, T=60) frames -> temporal squeeze-excite attention over T -> multi-step
IF neuron (integrate, threshold 0.6, hard reset) emitting binary spikes.

Sharding: data-parallel over batch B across 8 NeuronCores (1 batch element
per core); conv weights replicated. Everything on-device is fp32 (the
thresholded spike output is brittle to low-precision conv error).

Conv formulation (fastest measured fp32 shape on this PE): each timestep pair
is one [128, 1156] DMA with the two zero-padded images stacked on the
partition axis; the 3x3 conv is 9 shifted K=64 matmuls per image-half
accumulated in PSUM, with the two images issued interleaved on opposite PE
row strips (tile_position 0/64) so their fp32 passes overlap. fp32 matmul on
TRN2 lowers to 2 HW passes; measured steady state is ~429ns per logical
[64,128,512] matmul pair-slot.

Timesteps 0..R_RES-1 keep conv output resident in SBUF f32; the tail is
spilled to DRAM in phase 1 and prefetched back during the scan. The IF scan
runs 2 fused DVE scalar_tensor_tensor ops per step (integrate+attention in
one, threshold-reset in one), with the exact spike threshold computed on the
Scalar engine as Sign(Relu(u - nextafter(0.6))), which matches (u >= 0.6)
bit-exactly including ties.
"""

import sys

sys.path.insert(0, "/opt/trn_rl_repo")

import numpy as np
from contextlib import ExitStack

import concourse.bass as bass
import concourse.mybir as mybir
import concourse.tile as tile
from concourse.vector_clock import ScopedClock
from concourse.bass_utils import run_bass_kernel_spmd

B, T, CIN, H, W = 8, 60, 64, 32, 32
COUT = 128
TR = 3
HP, WP = H + 2, W + 2  # zero-padded spatial dims (34x34), padding done on host
NPAD = HP * WP  # 1156
NPIX = H * W  # 1024
V_TH = 0.6
# largest f32 strictly below V_TH; u >= VTH  <=>  relu(u - V_TH_MINUS) > 0
V_TH_MINUS = float(np.nextafter(np.float32(V_TH), np.float32(-np.inf)))
N_CORES = 8
R_RES = 30  # timesteps whose conv output stays resident in SBUF f32
THRESH_ON_ACT = True  # spike threshold via ACT Sign(Relu(.)) vs DVE is_ge
CONV_F32R = False  # use reduced-precision float32r matmuls for the conv
SCAN_SPLIT = 0  # 0 = whole-width scan on DVE; else DVE gets cols [0, SCAN_SPLIT)
F32 = mybir.dt.float32
ALU = mybir.AluOpType
ACTF = mybir.ActivationFunctionType

_drain_patched = False
_tjb_patched = False


def _legalize_single_wait(bir: bytes) -> bytes:
    """This walrus build allows at most ONE sync-wait per instruction, but the
    Tile scheduler attaches several. Hoist all but one wait of each instruction
    into single-wait EventSemaphore preludes on the same engine (same-engine
    program order preserves semantics)."""
    import orjson

    j = orjson.loads(bir)
    n = 0
    for f in j["functions"]:
        for bb in f["blocks"]:
            insts = bb.get("instructions") or []
            if not any(
                len((i.get("sync_info") or {}).get("on_wait") or []) > 1 for i in insts
            ):
                continue
            out = []
            for ins in insts:
                si = ins.get("sync_info") or {}
                waits = si.get("on_wait") or []
                if len(waits) > 1:
                    for wx in waits[:-1]:
                        n += 1
                        out.append(
                            {
                                "debug": ins.get("debug", 0),
                                "engine": ins["engine"],
                                "ins": [],
                                "name": f"wsplit-{n}",
                                "opcode": "EventSemaphore",
                                "outs": [],
                                "sync_info": {"on_update": [], "on_wait": [wx]},
                            }
                        )
                    si["on_wait"] = [waits[-1]]
                out.append(ins)
            bb["instructions"] = out
    return orjson.dumps(j)


def _patch_to_json_bytes():
    global _tjb_patched
    if _tjb_patched:
        return
    _tjb_patched = True
    orig = bass.Bass.to_json_bytes
    bass.Bass.to_json_bytes = lambda self: _legalize_single_wait(orig(self))


def _patch_tile_drain():
    """This walrus build allows only one sync-wait per CTRL instruction, but
    TileContext._drain_and_barrier puts every outstanding proc's wait on a
    single tail Drain. Split the waits across single-wait NOPs."""
    global _drain_patched
    if _drain_patched:
        return
    _drain_patched = True

    def _drain_and_barrier(self, tick_clock, wait_clock):
        gc = tick_clock.global_clock
        for proc in range(len(gc)):
            tick = gc[proc]
            if tick <= 0:
                continue
            sc = ScopedClock()
            sc.require_at_least(None, proc, tick)
            w = self.nc.sync.nop(nofuse=True)
            wait_clock.add_sem_waits(w.ins, sc)
        self.nc.sync.drain()
        self.nc.all_engine_barrier()
        popped = self.nc._tile_sem_poison_stack.pop()
        assert popped is self._sem_poison
        self.nc.clear_and_free_semaphores(list(self.sems.allocated().values()))
        self.nc.all_engine_barrier()

    tile.TileContext._drain_and_barrier = _drain_and_barrier


def build_program():
    _patch_tile_drain()
    _patch_to_json_bytes()
    nc = bass.Bass("TRN2", target_bir_lowering=False, debug=False, num_devices=N_CORES)

    CDT = mybir.dt.float32r if CONV_F32R else F32
    # Timestep PAIRS: two padded 64-channel images stacked on the partition
    # axis -> one [128, 1156] DMA per pair; the two images run as concurrent
    # K=64 matmuls on opposite PE row strips (fastest measured fp32 shape).
    x_d = nc.declare_dram_parameter("x", [T // 2, 2 * CIN, NPAD], CDT, isOutput=False)
    # 9 taps as lhsT [ci, co], duplicated on both partition halves.
    w_d = nc.declare_dram_parameter("w", [2 * CIN, 9 * COUT], CDT, isOutput=False)
    b_d = nc.declare_dram_parameter("bias", [COUT, 1], F32, isOutput=False)
    w1t_d = nc.declare_dram_parameter("w1t", [T, TR], F32, isOutput=False)
    w2t_d = nc.declare_dram_parameter("w2t", [TR, T], F32, isOutput=False)
    ones_d = nc.declare_dram_parameter("ones", [COUT, 1], F32, isOutput=False)
    onesr_d = nc.declare_dram_parameter("onesr", [1, 128], F32, isOutput=False)
    id_d = nc.declare_dram_parameter("ident", [128, 128], F32, isOutput=False)
    spk_d = nc.declare_dram_parameter("spk", [T, COUT, NPIX], F32, isOutput=True)

    yspill_d = nc.dram_tensor("yspill", [T - R_RES, COUT, NPIX], F32)

    with ExitStack() as ctx:
        tc = ctx.enter_context(tile.TileContext(nc))

        consts = ctx.enter_context(tc.tile_pool(name="consts", bufs=1))
        xpool = ctx.enter_context(tc.tile_pool(name="xpool", bufs=3))
        respool = ctx.enter_context(tc.tile_pool(name="respool", bufs=1))
        yscr = ctx.enter_context(tc.tile_pool(name="yscr", bufs=4))
        upool = ctx.enter_context(tc.tile_pool(name="upool", bufs=2))
        vpool = ctx.enter_context(tc.tile_pool(name="vpool", bufs=1))
        rpool = ctx.enter_context(tc.tile_pool(name="rpool", bufs=2))
        spool = ctx.enter_context(tc.tile_pool(name="spool", bufs=4))
        stats = ctx.enter_context(tc.tile_pool(name="stats", bufs=1))
        psum = ctx.enter_context(tc.tile_pool(name="psum", bufs=4, space="PSUM"))

        # --- load constants/weights ---
        w_t = consts.tile([2 * CIN, 9 * COUT], CDT)
        nc.sync.dma_start(w_t[:], w_d[:])
        b_t = consts.tile([COUT, 1], F32)
        nc.sync.dma_start(b_t[:], b_d[:])
        w1t_t = consts.tile([T, TR], F32)
        nc.sync.dma_start(w1t_t[:], w1t_d[:])
        w2t_t = consts.tile([TR, T], F32)
        nc.sync.dma_start(w2t_t[:], w2t_d[:])
        ones_t = consts.tile([COUT, 1], F32)
        nc.sync.dma_start(ones_t[:], ones_d[:])
        onesr_t = consts.tile([1, 128], F32)
        nc.sync.dma_start(onesr_t[:], onesr_d[:])
        id_t = consts.tile([128, 128], F32)
        nc.sync.dma_start(id_t[:], id_d[:])

        sums_t = stats.tile([COUT, T], F32)
        maxs_t = stats.tile([COUT, T], F32)
        thm_t = stats.tile([COUT, 1], F32)
        nc.vector.memset(thm_t[:], -V_TH_MINUS)

        res_y = respool.tile([COUT, R_RES * NPIX], F32)

        def conv_pair(p):
            """Emit conv for timestep pair p (t=2p, 2p+1). The two images sit
            on opposite partition halves and run as concurrent K=64 matmuls on
            opposite PE row strips. Returns the two PSUM tiles."""
            x_t = xpool.tile([2 * CIN, NPAD], CDT, tag="x", name="x")
            nc.sync.dma_start(x_t[:], x_d[p])
            xv = x_t[:].rearrange("p (h w) -> p h w", h=HP, w=WP)
            pys = [psum.tile([COUT, NPIX], F32, tag="py", name="py") for _ in range(2)]
            for half in range(2):
                h0 = half * 16
                outs = [
                    pys[img][:, half * 512 : (half + 1) * 512].rearrange(
                        "p (h w) -> p h w", h=16, w=W
                    )
                    for img in range(2)
                ]
                for o in range(9):
                    kh, kw = o // 3, o % 3
                    for img in range(2):
                        rhs = xv[
                            img * CIN : (img + 1) * CIN,
                            h0 + kh : h0 + kh + 16,
                            kw : kw + W,
                        ]
                        lhsT = w_t[
                            img * CIN : (img + 1) * CIN, o * COUT : (o + 1) * COUT
                        ]
                        nc.tensor.matmul(
                            outs[img],
                            lhsT,
                            rhs,
                            start=(o == 0),
                            stop=(o == 8),
                            tile_position=(img * CIN, 0),
                        )
            return pys

        # --- phase 1: conv all t; stats; t < R_RES resident, rest spilled ---
        for p in range(T // 2):
            pys = conv_pair(p)
            for img in range(2):
                t = 2 * p + img
                if t < R_RES:
                    y_sb = res_y[:, t * NPIX : (t + 1) * NPIX]
                else:
                    y_sb = yscr.tile([COUT, NPIX], F32, tag="ys", name="ys")[:]
                nc.scalar.activation(
                    y_sb,
                    pys[img][:],
                    ACTF.Identity,
                    bias=b_t[:, 0:1],
                    accum_out=sums_t[:, t : t + 1],
                )
                nc.vector.tensor_reduce(
                    maxs_t[:, t : t + 1], y_sb, mybir.AxisListType.X, ALU.max
                )
                if t >= R_RES:
                    nc.sync.dma_start(yspill_d[t - R_RES], y_sb)

        # --- phase B: temporal attention (tiny) ---
        pavg_ps = psum.tile([T, 1], F32, tag="py", name="pavg_ps")
        nc.tensor.matmul(pavg_ps[:], sums_t[:], ones_t[:], start=True, stop=True)
        maxT_ps = psum.tile([T, 128], F32, tag="py", name="maxT_ps")
        nc.tensor.transpose(maxT_ps[:], maxs_t[:], id_t[:])
        pcat = stats.tile([T, 2], F32)
        nc.vector.tensor_copy(pcat[:, 0:1], pavg_ps[:])
        nc.vector.tensor_reduce(
            pcat[:, 1:2], maxT_ps[:], mybir.AxisListType.X, ALU.max
        )
        z1_ps = psum.tile([TR, 2], F32, tag="py", name="z1_ps")
        nc.tensor.matmul(z1_ps[:], w1t_t[:], pcat[:], start=True, stop=True)
        r1 = stats.tile([TR, 2], F32)
        nc.scalar.activation(r1[:], z1_ps[:], ACTF.Relu)
        z2_ps = psum.tile([1, T], F32, tag="py", name="z2_ps")
        nc.tensor.matmul(z2_ps[:], r1[:, 0:1], w2t_t[:], start=True, stop=False)
        nc.tensor.matmul(z2_ps[:], r1[:, 1:2], w2t_t[:], start=False, stop=True)
        att_row = stats.tile([1, T], F32)
        nc.scalar.activation(att_row[:], z2_ps[:], ACTF.Sigmoid)
        attB_ps = psum.tile([COUT, T], F32, tag="py", name="attB_ps")
        nc.tensor.matmul(attB_ps[:], onesr_t[:], att_row[:], start=True, stop=True)
        attB = stats.tile([COUT, T], F32)
        nc.vector.tensor_copy(attB[:], attB_ps[:])

        # --- phase 2a: prefetch spilled y back (no att dependency) ---
        scratch = {}
        for t in range(R_RES, T):
            yld = yscr.tile([COUT, NPIX], F32, tag="ys", name="ys")
            nc.sync.dma_start(yld[:], yspill_d[t - R_RES])
            scratch[t] = yld

        # --- phase 2b: IF scan over T ---
        if not SCAN_SPLIT:
            v_t = vpool.tile([COUT, NPIX], F32)
            for t in range(T):
                if t < R_RES:
                    ysrc = res_y[:, t * NPIX : (t + 1) * NPIX]
                else:
                    ysrc = scratch[t][:]
                u = upool.tile([COUT, NPIX], F32, tag="u", name="u")
                if t == 0:
                    nc.vector.tensor_scalar(
                        u[:], ysrc, attB[:, t : t + 1], None, ALU.mult
                    )
                else:
                    nc.vector.scalar_tensor_tensor(
                        u[:], ysrc, attB[:, t : t + 1], v_t[:], ALU.mult, ALU.add
                    )
                s = spool.tile([COUT, NPIX], F32, tag="s", name="s")
                if THRESH_ON_ACT:
                    r = rpool.tile([COUT, NPIX], F32, tag="r", name="r")
                    nc.scalar.activation(r[:], u[:], ACTF.Relu, bias=thm_t[:, 0:1])
                    nc.scalar.activation(s[:], r[:], ACTF.Sign)
                else:
                    nc.vector.tensor_scalar(s[:], u[:], V_TH, None, ALU.is_ge)
                nc.vector.scalar_tensor_tensor(
                    v_t[:], u[:], V_TH, u[:], ALU.is_lt, ALU.mult
                )
                nc.sync.dma_start(spk_d[t], s[:])
        else:
            # Split the hw dimension: cols [0, NA) run their chain on DVE,
            # cols [NA, NPIX) on GpSimd — two independent recurrences in
            # parallel. Threshold: A-half DVE is_ge (2x mode), B-half ACT.
            NA = SCAN_SPLIT
            NB = NPIX - NA
            vA = vpool.tile([COUT, NA], F32, tag="vA", name="vA")
            vB = vpool.tile([COUT, NB], F32, tag="vB", name="vB")
            for t in range(T):
                if t < R_RES:
                    yA = res_y[:, t * NPIX : t * NPIX + NA]
                    yB = res_y[:, t * NPIX + NA : (t + 1) * NPIX]
                else:
                    yA = scratch[t][:, 0:NA]
                    yB = scratch[t][:, NA:NPIX]
                att = attB[:, t : t + 1]
                uA = upool.tile([COUT, NA], F32, tag="uA", name="uA")
                uB = upool.tile([COUT, NB], F32, tag="uB", name="uB")
                if t == 0:
                    nc.vector.tensor_scalar(uA[:], yA, att, None, ALU.mult)
                    nc.gpsimd.tensor_scalar(uB[:], yB, att, None, ALU.mult)
                else:
                    nc.vector.scalar_tensor_tensor(
                        uA[:], yA, att, vA[:], ALU.mult, ALU.add
                    )
                    nc.gpsimd.scalar_tensor_tensor(
                        uB[:], yB, att, vB[:], ALU.mult, ALU.add
                    )
                sA = spool.tile([COUT, NA], F32, tag="sA", name="sA")
                sB = spool.tile([COUT, NB], F32, tag="sB", name="sB")
                nc.vector.tensor_scalar(sA[:], uA[:], V_TH, None, ALU.is_ge)
                rB = rpool.tile([COUT, NB], F32, tag="rB", name="rB")
                nc.scalar.activation(rB[:], uB[:], ACTF.Relu, bias=thm_t[:, 0:1])
                nc.scalar.activation(sB[:], rB[:], ACTF.Sign)
                nc.vector.scalar_tensor_tensor(
                    vA[:], uA[:], V_TH, uA[:], ALU.is_lt, ALU.mult
                )
                nc.gpsimd.scalar_tensor_tensor(
                    vB[:], uB[:], V_TH, uB[:], ALU.is_lt, ALU.mult
                )
                nc.sync.dma_start(spk_d[t][:, 0:NA], sA[:])
                nc.sync.dma_start(spk_d[t][:, NA:NPIX], sB[:])

    return nc


def prep_inputs(data, conv_w, conv_b, ta_w1, ta_w2):
    data = np.ascontiguousarray(np.asarray(data, dtype=np.float32))
    conv_w = np.asarray(conv_w, dtype=np.float32)
    conv_b = np.asarray(conv_b, dtype=np.float32)
    ta_w1 = np.asarray(ta_w1, dtype=np.float32)
    ta_w2 = np.asarray(ta_w2, dtype=np.float32)

    xpad = np.zeros((B, T, CIN, HP, WP), np.float32)
    xpad[:, :, :, 1 : H + 1, 1 : W + 1] = data
    xc = xpad.reshape(B, T // 2, 2 * CIN, NPAD)

    wmat = conv_w.transpose(1, 2, 3, 0).reshape(CIN, 9, COUT)  # [ci, kh*3+kw, co]
    w_in = np.empty((2 * CIN, 9 * COUT), np.float32)
    for o in range(9):
        w_in[0:CIN, o * COUT : (o + 1) * COUT] = wmat[:, o, :]
        w_in[CIN:, o * COUT : (o + 1) * COUT] = wmat[:, o, :]

    aux = {
        "w": w_in,
        "bias": conv_b.reshape(COUT, 1),
        "w1t": np.ascontiguousarray(ta_w1.T),
        "w2t": np.ascontiguousarray(ta_w2.T),
        "ones": np.full((COUT, 1), 1.0 / (COUT * NPIX), np.float32),
        "onesr": np.ones((1, 128), np.float32),
        "ident": np.eye(128, dtype=np.float32),
    }
    return [{"x": np.ascontiguousarray(xc[b]), **aux} for b in range(B)]


def kernel(data, conv_w, conv_b, ta_w1, ta_w2):
    in_maps = prep_inputs(data, conv_w, conv_b, ta_w1, ta_w2)
    nc = build_program()
    res = run_bass_kernel_spmd(nc, in_maps, list(range(N_CORES)))
    out = np.stack(
        [res.results[b]["spk"].reshape(T, COUT, H, W) for b in range(B)], axis=0
    )
    return np.ascontiguousarray(out.astype(np.float32))



# revision 7
# speedup vs baseline: 1.2059x; 1.2059x over previous
"""Trainium2 Bass kernel for nn_ConvAttLIF: Conv2d(64->128, 3x3, pad1) over
(B=8, T=60) frames -> temporal squeeze-excite attention over T -> multi-step
IF neuron (integrate, threshold 0.6, hard reset) emitting binary spikes.

Sharding: data-parallel over batch B across 8 NeuronCores (1 batch element
per core); conv weights replicated.

Conv formulation: each timestep pair is one [128, 1156] DMA with the two
zero-padded images stacked on the partition axis; the 3x3 conv is 9 shifted
K=64 matmuls per image-half accumulated in PSUM, with the two images issued
interleaved on opposite PE row strips (tile_position 0/64).

Precision scheme: the main conv matmuls run in float32r (1 cycle/row vs
fp32's 4), which rounds BOTH operands to ~11 mantissa bits (RN, measured on
HW). The resulting spike flips exceed the 2e-2 gate, so bf16 correction
matmuls accumulate the known rounding residuals into the same PSUM group:
  W-corr (all timesteps):  bf16(x) @ bf16(w - R11(w))
  X-corr (XCORR_FRAC of pairs): bf16(x - R11(x)) @ bf16(w)
Each correction pass costs 1 cycle/row, so conv = (2 + frac) cycles/row.

The IF scan is column-split across engines: DVE runs the u/v chain for cols
[0, CD), GpSimd for [CD, 1024); ACT computes spikes as Sign(u - theta') into
fp8 (values {-1, 0, +1}); host maps (raw > 0) -> {0, 1}. The spill tail of
y (t >= R_RES) round-trips DRAM in fp32, prefetched back during the scan.
"""

import sys

sys.path.insert(0, "/opt/trn_rl_repo")

import numpy as np
from contextlib import ExitStack

import concourse.bass as bass
import concourse.mybir as mybir
import concourse.tile as tile
from concourse.vector_clock import ScopedClock
from concourse.bass_utils import run_bass_kernel_spmd

B, T, CIN, H, W = 8, 60, 64, 32, 32
COUT = 128
TR = 3
HP, WP = H + 2, W + 2  # zero-padded spatial dims (34x34), padding done on host
NPAD = HP * WP  # 1156
NPIX = H * W  # 1024
V_TH = 0.6
# largest f32 strictly below V_TH; u >= VTH  <=>  u - V_TH_MINUS > 0
V_TH_MINUS = float(np.nextafter(np.float32(V_TH), np.float32(-np.inf)))
N_CORES = 8
R_RES = 32  # timesteps whose conv output stays resident in SBUF f32
RBITS = 11  # fp32r internal operand rounding: mantissa fraction bits kept
ROUND_RN = True  # True: round-to-nearest-even at RBITS; False: truncate
WCORR = True  # bf16 correction of w-side fp32r rounding (all pairs)
XCORR_FRAC = 1.0  # fraction of timestep pairs getting x-side correction
XCORR_HEAD = True  # cluster corrected pairs at the head (errors cascade forward)
# scan: DVE gets cols [0, CD) via 2 fused stt ops; GpSimd (Pool) gets cols
# [CD, NPIX) via a 4-op ts/tt decomposition (Pool rejects stt). CD=NPIX
# disables the Pool lane.
CD = 1024
F32 = mybir.dt.float32
F32R = mybir.dt.float32r
BF16 = mybir.dt.bfloat16
FP8 = mybir.dt.float8e4
ALU = mybir.AluOpType
ACTF = mybir.ActivationFunctionType

NPAIR = T // 2
# Subset of pairs receiving the X-side correction: head-clustered (early
# errors cascade through the whole scan; tail errors die quickly) or
# Bresenham-interleaved.
if XCORR_HEAD:
    XCORR_PAIRS = frozenset(range(int(round(NPAIR * XCORR_FRAC))))
else:
    XCORR_PAIRS = frozenset(
        p for p in range(NPAIR)
        if int((p + 1) * XCORR_FRAC) > int(p * XCORR_FRAC)
    )

_drain_patched = False
_tjb_patched = False


def _legalize_single_wait(bir: bytes) -> bytes:
    """This walrus build allows at most ONE sync-wait per instruction, but the
    Tile scheduler attaches several. Hoist all but one wait of each instruction
    into single-wait EventSemaphore preludes on the same engine (same-engine
    program order preserves semantics)."""
    import orjson

    j = orjson.loads(bir)
    n = 0
    for f in j["functions"]:
        for bb in f["blocks"]:
            insts = bb.get("instructions") or []
            if not any(
                len((i.get("sync_info") or {}).get("on_wait") or []) > 1 for i in insts
            ):
                continue
            out = []
            for ins in insts:
                si = ins.get("sync_info") or {}
                waits = si.get("on_wait") or []
                if len(waits) > 1:
                    for wx in waits[:-1]:
                        n += 1
                        out.append(
                            {
                                "debug": ins.get("debug", 0),
                                "engine": ins["engine"],
                                "ins": [],
                                "name": f"wsplit-{n}",
                                "opcode": "EventSemaphore",
                                "outs": [],
                                "sync_info": {"on_update": [], "on_wait": [wx]},
                            }
                        )
                    si["on_wait"] = [waits[-1]]
                out.append(ins)
            bb["instructions"] = out
    return orjson.dumps(j)


def _patch_to_json_bytes():
    global _tjb_patched
    if _tjb_patched:
        return
    _tjb_patched = True
    orig = bass.Bass.to_json_bytes
    bass.Bass.to_json_bytes = lambda self: _legalize_single_wait(orig(self))


def _patch_tile_drain():
    """This walrus build allows only one sync-wait per CTRL instruction, but
    TileContext._drain_and_barrier puts every outstanding proc's wait on a
    single tail Drain. Split the waits across single-wait NOPs."""
    global _drain_patched
    if _drain_patched:
        return
    _drain_patched = True

    def _drain_and_barrier(self, tick_clock, wait_clock):
        gc = tick_clock.global_clock
        for proc in range(len(gc)):
            tick = gc[proc]
            if tick <= 0:
                continue
            sc = ScopedClock()
            sc.require_at_least(None, proc, tick)
            w = self.nc.sync.nop(nofuse=True)
            wait_clock.add_sem_waits(w.ins, sc)
        self.nc.sync.drain()
        self.nc.all_engine_barrier()
        popped = self.nc._tile_sem_poison_stack.pop()
        assert popped is self._sem_poison
        self.nc.clear_and_free_semaphores(list(self.sems.allocated().values()))
        self.nc.all_engine_barrier()

    tile.TileContext._drain_and_barrier = _drain_and_barrier


def build_program():
    _patch_tile_drain()
    _patch_to_json_bytes()
    nc = bass.Bass("TRN2", target_bir_lowering=False, debug=False, num_devices=N_CORES)

    # Timestep PAIRS: two padded 64-channel images stacked on the partition
    # axis -> one [128, 1156] DMA per pair; the two images run as concurrent
    # K=64 matmuls on opposite PE row strips.
    x_d = nc.declare_dram_parameter("x", [NPAIR, 2 * CIN, NPAD], F32R, isOutput=False)
    xb_d = nc.declare_dram_parameter("xb", [NPAIR, 2 * CIN, NPAD], BF16, isOutput=False)
    exb_d = nc.declare_dram_parameter(
        "exb", [NPAIR, 2 * CIN, NPAD], BF16, isOutput=False
    )
    # 9 taps as lhsT [ci, co], duplicated on both partition halves.
    w_d = nc.declare_dram_parameter("w", [2 * CIN, 9 * COUT], F32R, isOutput=False)
    ewb_d = nc.declare_dram_parameter("ewb", [2 * CIN, 9 * COUT], BF16, isOutput=False)
    wb_d = nc.declare_dram_parameter("wb", [2 * CIN, 9 * COUT], BF16, isOutput=False)
    b_d = nc.declare_dram_parameter("bias", [COUT, 1], F32, isOutput=False)
    w1t_d = nc.declare_dram_parameter("w1t", [T, TR], F32, isOutput=False)
    w2t_d = nc.declare_dram_parameter("w2t", [TR, T], F32, isOutput=False)
    ones_d = nc.declare_dram_parameter("ones", [COUT, 1], F32, isOutput=False)
    onesr_d = nc.declare_dram_parameter("onesr", [1, 128], F32, isOutput=False)
    id_d = nc.declare_dram_parameter("ident", [128, 128], F32, isOutput=False)
    spk_d = nc.declare_dram_parameter("spk", [T, COUT, NPIX], FP8, isOutput=True)

    yspill_d = nc.dram_tensor("yspill", [T - R_RES, COUT, NPIX], F32)

    CG = NPIX - CD

    with ExitStack() as ctx:
        tc = ctx.enter_context(tile.TileContext(nc))

        consts = ctx.enter_context(tc.tile_pool(name="consts", bufs=1))
        xpool = ctx.enter_context(tc.tile_pool(name="xpool", bufs=2))
        respool = ctx.enter_context(tc.tile_pool(name="respool", bufs=1))
        yscr = ctx.enter_context(tc.tile_pool(name="yscr", bufs=4))
        upool = ctx.enter_context(tc.tile_pool(name="upool", bufs=2))
        vpool = ctx.enter_context(tc.tile_pool(name="vpool", bufs=1))
        spool = ctx.enter_context(tc.tile_pool(name="spool", bufs=3))
        stats = ctx.enter_context(tc.tile_pool(name="stats", bufs=1))
        psum = ctx.enter_context(tc.tile_pool(name="psum", bufs=4, space="PSUM"))

        # --- load constants/weights ---
        w_t = consts.tile([2 * CIN, 9 * COUT], F32R)
        nc.sync.dma_start(w_t[:], w_d[:])
        ewb_t = consts.tile([2 * CIN, 9 * COUT], BF16)
        nc.sync.dma_start(ewb_t[:], ewb_d[:])
        wb_t = consts.tile([2 * CIN, 9 * COUT], BF16)
        nc.sync.dma_start(wb_t[:], wb_d[:])
        b_t = consts.tile([COUT, 1], F32)
        nc.sync.dma_start(b_t[:], b_d[:])
        w1t_t = consts.tile([T, TR], F32)
        nc.sync.dma_start(w1t_t[:], w1t_d[:])
        w2t_t = consts.tile([TR, T], F32)
        nc.sync.dma_start(w2t_t[:], w2t_d[:])
        ones_t = consts.tile([COUT, 1], F32)
        nc.sync.dma_start(ones_t[:], ones_d[:])
        onesr_t = consts.tile([1, 128], F32)
        nc.sync.dma_start(onesr_t[:], onesr_d[:])
        id_t = consts.tile([128, 128], F32)
        nc.sync.dma_start(id_t[:], id_d[:])

        sums_t = stats.tile([COUT, T], F32)
        maxs_t = stats.tile([COUT, T], F32)
        nthm_t = stats.tile([COUT, 1], F32)
        nc.vector.memset(nthm_t[:], -V_TH_MINUS)

        res_y = respool.tile([COUT, R_RES * NPIX], F32)

        def conv_pair(p):
            """Emit the full-precision conv for timestep pair p (t=2p, 2p+1):
            fp32r main pass + bf16 correction passes, all accumulating into
            the same PSUM tiles. Returns the two PSUM tiles."""
            x_t = xpool.tile([2 * CIN, NPAD], F32R, tag="x", name="x")
            nc.sync.dma_start(x_t[:], x_d[p])
            xviews = [x_t[:].rearrange("p (h w) -> p h w", h=HP, w=WP)]
            lhss = [w_t]
            if WCORR:
                xb_t = xpool.tile([2 * CIN, NPAD], BF16, tag="xb", name="xb")
                nc.sync.dma_start(xb_t[:], xb_d[p])
                xviews.append(xb_t[:].rearrange("p (h w) -> p h w", h=HP, w=WP))
                lhss.append(ewb_t)
            if p in XCORR_PAIRS:
                exb_t = xpool.tile([2 * CIN, NPAD], BF16, tag="exb", name="exb")
                nc.sync.dma_start(exb_t[:], exb_d[p])
                xviews.append(exb_t[:].rearrange("p (h w) -> p h w", h=HP, w=WP))
                lhss.append(wb_t)
            npass = len(lhss)
            pys = [psum.tile([COUT, NPIX], F32, tag="py", name="py") for _ in range(2)]
            for half in range(2):
                h0 = half * 16
                outs = [
                    pys[img][:, half * 512 : (half + 1) * 512].rearrange(
                        "p (h w) -> p h w", h=16, w=W
                    )
                    for img in range(2)
                ]
                for ps in range(npass):
                    xv = xviews[ps]
                    wsrc = lhss[ps]
                    for o in range(9):
                        kh, kw = o // 3, o % 3
                        for img in range(2):
                            rhs = xv[
                                img * CIN : (img + 1) * CIN,
                                h0 + kh : h0 + kh + 16,
                                kw : kw + W,
                            ]
                            lhsT = wsrc[
                                img * CIN : (img + 1) * CIN, o * COUT : (o + 1) * COUT
                            ]
                            nc.tensor.matmul(
                                outs[img],
                                lhsT,
                                rhs,
                                start=(ps == 0 and o == 0),
                                stop=(ps == npass - 1 and o == 8),
                                tile_position=(img * CIN, 0),
                            )
            return pys

        # --- phase 1: conv all t; stats; t < R_RES resident, rest spilled ---
        for p in range(NPAIR):
            pys = conv_pair(p)
            for img in range(2):
                t = 2 * p + img
                if t < R_RES:
                    y_sb = res_y[:, t * NPIX : (t + 1) * NPIX]
                else:
                    y_sb = yscr.tile([COUT, NPIX], F32, tag="ys", name="ys")[:]
                nc.scalar.activation(
                    y_sb,
                    pys[img][:],
                    ACTF.Identity,
                    bias=b_t[:, 0:1],
                    accum_out=sums_t[:, t : t + 1],
                )
                nc.vector.tensor_reduce(
                    maxs_t[:, t : t + 1], y_sb, mybir.AxisListType.X, ALU.max
                )
                if t >= R_RES:
                    nc.sync.dma_start(yspill_d[t - R_RES], y_sb)

        # --- phase B: temporal attention (tiny) ---
        pavg_ps = psum.tile([T, 1], F32, tag="py", name="pavg_ps")
        nc.tensor.matmul(pavg_ps[:], sums_t[:], ones_t[:], start=True, stop=True)
        maxT_ps = psum.tile([T, 128], F32, tag="py", name="maxT_ps")
        nc.tensor.transpose(maxT_ps[:], maxs_t[:], id_t[:])
        pcat = stats.tile([T, 2], F32)
        nc.vector.tensor_copy(pcat[:, 0:1], pavg_ps[:])
        nc.vector.tensor_reduce(
            pcat[:, 1:2], maxT_ps[:], mybir.AxisListType.X, ALU.max
        )
        z1_ps = psum.tile([TR, 2], F32, tag="py", name="z1_ps")
        nc.tensor.matmul(z1_ps[:], w1t_t[:], pcat[:], start=True, stop=True)
        r1 = stats.tile([TR, 2], F32)
        nc.scalar.activation(r1[:], z1_ps[:], ACTF.Relu)
        z2_ps = psum.tile([1, T], F32, tag="py", name="z2_ps")
        nc.tensor.matmul(z2_ps[:], r1[:, 0:1], w2t_t[:], start=True, stop=False)
        nc.tensor.matmul(z2_ps[:], r1[:, 1:2], w2t_t[:], start=False, stop=True)
        att_row = stats.tile([1, T], F32)
        nc.scalar.activation(att_row[:], z2_ps[:], ACTF.Sigmoid)
        attB_ps = psum.tile([COUT, T], F32, tag="py", name="attB_ps")
        nc.tensor.matmul(attB_ps[:], onesr_t[:], att_row[:], start=True, stop=True)
        attB = stats.tile([COUT, T], F32)
        nc.vector.tensor_copy(attB[:], attB_ps[:])

        # --- phase 2a: prefetch spilled y back (no att dependency) ---
        scratch = {}
        for t in range(R_RES, T):
            yld = yscr.tile([COUT, NPIX], F32, tag="ys", name="ys")
            nc.sync.dma_start(yld[:], yspill_d[t - R_RES])
            scratch[t] = yld

        # --- phase 2b: IF scan over T on DVE (2 stt/step); spikes on ACT ---
        # (GpSimd/Pool is useless here: it rejects stt, its tensor_scalar
        # ucode is ~15us/op, and concurrent Pool ops destroy DVE throughput
        # via the shared SBUF ports.)
        vD = vpool.tile([COUT, NPIX], F32)
        for t in range(T):
            if t < R_RES:
                ysrc = res_y[:, t * NPIX : (t + 1) * NPIX]
            else:
                ysrc = scratch[t][:]
            att = attB[:, t : t + 1]
            uD = upool.tile([COUT, NPIX], F32, tag="uD", name="uD")
            if t == 0:
                nc.vector.tensor_scalar(uD[:], ysrc, att, None, ALU.mult)
            else:
                nc.vector.scalar_tensor_tensor(
                    uD[:], ysrc, att, vD[:], ALU.mult, ALU.add
                )
            sD = spool.tile([COUT, NPIX], FP8, tag="sD", name="sD")
            nc.scalar.activation(sD[:], uD[:], ACTF.Sign, bias=nthm_t[:, 0:1])
            nc.vector.scalar_tensor_tensor(
                vD[:], uD[:], V_TH, uD[:], ALU.is_lt, ALU.mult
            )
            nc.sync.dma_start(spk_d[t], sD[:])

    return nc


def _rhat(a):
    """Host model of the fp32r internal operand rounding."""
    a = np.asarray(a, np.float32)
    ai = a.view(np.uint32)
    if ROUND_RN:
        au = ai.astype(np.uint64)
        shift = np.uint64(23 - RBITS)
        one = np.uint64(1)
        half = one << (shift - one)
        lsb = (au >> shift) & one
        out = ((au + half - one + lsb) >> shift << shift).astype(np.uint32)
    else:
        mask = np.uint32(0xFFFFFFFF) << np.uint32(23 - RBITS)
        out = ai & mask
    return out.view(np.float32)


def prep_inputs(data, conv_w, conv_b, ta_w1, ta_w2):
    import ml_dtypes

    bf16 = ml_dtypes.bfloat16
    data = np.ascontiguousarray(np.asarray(data, dtype=np.float32))
    conv_w = np.asarray(conv_w, dtype=np.float32)
    conv_b = np.asarray(conv_b, dtype=np.float32)
    ta_w1 = np.asarray(ta_w1, dtype=np.float32)
    ta_w2 = np.asarray(ta_w2, dtype=np.float32)

    xpad = np.zeros((B, T, CIN, HP, WP), np.float32)
    xpad[:, :, :, 1 : H + 1, 1 : W + 1] = data
    xc = xpad.reshape(B, NPAIR, 2 * CIN, NPAD)
    xc_b = xc.astype(bf16)
    exc_b = (xc - _rhat(xc)).astype(bf16)

    wmat = conv_w.transpose(1, 2, 3, 0).reshape(CIN, 9, COUT)  # [ci, kh*3+kw, co]
    w_in = np.empty((2 * CIN, 9 * COUT), np.float32)
    for o in range(9):
        w_in[0:CIN, o * COUT : (o + 1) * COUT] = wmat[:, o, :]
        w_in[CIN:, o * COUT : (o + 1) * COUT] = wmat[:, o, :]
    ew_in = (w_in - _rhat(w_in)).astype(bf16)
    wb_in = w_in.astype(bf16)

    aux = {
        "w": w_in,
        "ewb": ew_in,
        "wb": wb_in,
        "bias": conv_b.reshape(COUT, 1),
        "w1t": np.ascontiguousarray(ta_w1.T),
        "w2t": np.ascontiguousarray(ta_w2.T),
        "ones": np.full((COUT, 1), 1.0 / (COUT * NPIX), np.float32),
        "onesr": np.ones((1, 128), np.float32),
        "ident": np.eye(128, dtype=np.float32),
    }
    return [
        {
            "x": np.ascontiguousarray(xc[b]),
            "xb": np.ascontiguousarray(xc_b[b]),
            "exb": np.ascontiguousarray(exc_b[b]),
            **aux,
        }
        for b in range(B)
    ]


def spikes_from_raw(raw):
    """Map the fp8 Sign-domain output {-1, 0, +1} to {0.0, 1.0} float32."""
    raw = np.asarray(raw)
    if raw.dtype == np.uint8:
        import ml_dtypes

        raw = raw.view(ml_dtypes.float8_e4m3)
    return (raw.astype(np.float32) > 0).astype(np.float32)


def kernel(data, conv_w, conv_b, ta_w1, ta_w2):
    in_maps = prep_inputs(data, conv_w, conv_b, ta_w1, ta_w2)
    nc = build_program()
    res = run_bass_kernel_spmd(nc, in_maps, list(range(N_CORES)))
    out = np.stack(
        [
            spikes_from_raw(res.results[b]["spk"]).reshape(T, COUT, H, W)
            for b in range(B)
        ],
        axis=0,
    )
    return np.ascontiguousarray(out.astype(np.float32))


# revision 8
# speedup vs baseline: 1.3800x; 1.1444x over previous
"""Trainium2 Bass kernel for nn_ConvAttLIF: Conv2d(64->128, 3x3, pad1) over
(B=8, T=60) frames -> temporal squeeze-excite attention over T -> multi-step
IF neuron (integrate, threshold 0.6, hard reset) emitting binary spikes.

Sharding: data-parallel over batch B across 8 NeuronCores (1 batch element
per core); conv weights replicated.

Conv formulation: each timestep pair is one [128, 1156] DMA with the two
zero-padded images stacked on the partition axis; the 3x3 conv is 9 shifted
K=64 matmuls per image-half accumulated in PSUM, with the two images issued
interleaved on opposite PE row strips (tile_position 0/64).

Precision scheme: the main conv matmuls run in float32r (1 cycle/row vs
fp32's 4), which rounds BOTH operands to ~11 mantissa bits (RN, measured on
HW). The resulting spike flips exceed the 2e-2 gate, so bf16 correction
matmuls accumulate the known rounding residuals into the same PSUM group:
  W-corr (all timesteps):  bf16(x) @ bf16(w - R11(w))
  X-corr (XCORR_FRAC of pairs): bf16(x - R11(x)) @ bf16(w)
Each correction pass costs 1 cycle/row, so conv = (2 + frac) cycles/row.

The IF scan is column-split across engines: DVE runs the u/v chain for cols
[0, CD), GpSimd for [CD, 1024); ACT computes spikes as Sign(u - theta') into
fp8 (values {-1, 0, +1}); host maps (raw > 0) -> {0, 1}. The spill tail of
y (t >= R_RES) round-trips DRAM in fp32, prefetched back during the scan.
"""

import sys

sys.path.insert(0, "/opt/trn_rl_repo")

import numpy as np
from contextlib import ExitStack

import concourse.bass as bass
import concourse.mybir as mybir
import concourse.tile as tile
from concourse.vector_clock import ScopedClock
from concourse.bass_utils import run_bass_kernel_spmd

B, T, CIN, H, W = 8, 60, 64, 32, 32
COUT = 128
TR = 3
HP, WP = H + 2, W + 2  # zero-padded spatial dims (34x34), padding done on host
NPAD = HP * WP  # 1156
NPIX = H * W  # 1024
V_TH = 0.6
# largest f32 strictly below V_TH; u >= VTH  <=>  u - V_TH_MINUS > 0
V_TH_MINUS = float(np.nextafter(np.float32(V_TH), np.float32(-np.inf)))
N_CORES = 8
R_RES = 32  # timesteps whose conv output stays resident in SBUF f32
RBITS = 11  # fp32r internal operand rounding: mantissa fraction bits kept
ROUND_RN = True  # True: round-to-nearest-even at RBITS; False: truncate
WCORR = True  # bf16 correction of w-side fp32r rounding (all pairs)
XCORR_FRAC = 0.4  # fraction of timestep pairs getting x-side correction
XCORR_HEAD = True  # cluster corrected pairs at the head (errors cascade forward)
# scan: DVE gets cols [0, CD) via 2 fused stt ops; GpSimd (Pool) gets cols
# [CD, NPIX) via a 4-op ts/tt decomposition (Pool rejects stt). CD=NPIX
# disables the Pool lane.
CD = 1024
F32 = mybir.dt.float32
F32R = mybir.dt.float32r
BF16 = mybir.dt.bfloat16
FP8 = mybir.dt.float8e4
ALU = mybir.AluOpType
ACTF = mybir.ActivationFunctionType

NPAIR = T // 2
# Subset of pairs receiving the X-side correction: head-clustered (early
# errors cascade through the whole scan; tail errors die quickly) or
# Bresenham-interleaved.
if XCORR_HEAD:
    XCORR_PAIRS = frozenset(range(int(round(NPAIR * XCORR_FRAC))))
else:
    XCORR_PAIRS = frozenset(
        p for p in range(NPAIR)
        if int((p + 1) * XCORR_FRAC) > int(p * XCORR_FRAC)
    )

_drain_patched = False
_tjb_patched = False


def _legalize_single_wait(bir: bytes) -> bytes:
    """This walrus build allows at most ONE sync-wait per instruction, but the
    Tile scheduler attaches several. Hoist all but one wait of each instruction
    into single-wait EventSemaphore preludes on the same engine (same-engine
    program order preserves semantics)."""
    import orjson

    j = orjson.loads(bir)
    n = 0
    for f in j["functions"]:
        for bb in f["blocks"]:
            insts = bb.get("instructions") or []
            if not any(
                len((i.get("sync_info") or {}).get("on_wait") or []) > 1 for i in insts
            ):
                continue
            out = []
            for ins in insts:
                si = ins.get("sync_info") or {}
                waits = si.get("on_wait") or []
                if len(waits) > 1:
                    for wx in waits[:-1]:
                        n += 1
                        out.append(
                            {
                                "debug": ins.get("debug", 0),
                                "engine": ins["engine"],
                                "ins": [],
                                "name": f"wsplit-{n}",
                                "opcode": "EventSemaphore",
                                "outs": [],
                                "sync_info": {"on_update": [], "on_wait": [wx]},
                            }
                        )
                    si["on_wait"] = [waits[-1]]
                out.append(ins)
            bb["instructions"] = out
    return orjson.dumps(j)


def _patch_to_json_bytes():
    global _tjb_patched
    if _tjb_patched:
        return
    _tjb_patched = True
    orig = bass.Bass.to_json_bytes
    bass.Bass.to_json_bytes = lambda self: _legalize_single_wait(orig(self))


def _patch_tile_drain():
    """This walrus build allows only one sync-wait per CTRL instruction, but
    TileContext._drain_and_barrier puts every outstanding proc's wait on a
    single tail Drain. Split the waits across single-wait NOPs."""
    global _drain_patched
    if _drain_patched:
        return
    _drain_patched = True

    def _drain_and_barrier(self, tick_clock, wait_clock):
        gc = tick_clock.global_clock
        for proc in range(len(gc)):
            tick = gc[proc]
            if tick <= 0:
                continue
            sc = ScopedClock()
            sc.require_at_least(None, proc, tick)
            w = self.nc.sync.nop(nofuse=True)
            wait_clock.add_sem_waits(w.ins, sc)
        self.nc.sync.drain()
        self.nc.all_engine_barrier()
        popped = self.nc._tile_sem_poison_stack.pop()
        assert popped is self._sem_poison
        self.nc.clear_and_free_semaphores(list(self.sems.allocated().values()))
        self.nc.all_engine_barrier()

    tile.TileContext._drain_and_barrier = _drain_and_barrier


def build_program():
    _patch_tile_drain()
    _patch_to_json_bytes()
    nc = bass.Bass("TRN2", target_bir_lowering=False, debug=False, num_devices=N_CORES)

    # Timestep PAIRS: two padded 64-channel images stacked on the partition
    # axis -> one [128, 1156] DMA per pair; the two images run as concurrent
    # K=64 matmuls on opposite PE row strips.
    x_d = nc.declare_dram_parameter("x", [NPAIR, 2 * CIN, NPAD], F32R, isOutput=False)
    xb_d = nc.declare_dram_parameter("xb", [NPAIR, 2 * CIN, NPAD], BF16, isOutput=False)
    exb_d = nc.declare_dram_parameter(
        "exb", [NPAIR, 2 * CIN, NPAD], BF16, isOutput=False
    )
    # 9 taps as lhsT [ci, co], duplicated on both partition halves.
    w_d = nc.declare_dram_parameter("w", [2 * CIN, 9 * COUT], F32R, isOutput=False)
    ewb_d = nc.declare_dram_parameter("ewb", [2 * CIN, 9 * COUT], BF16, isOutput=False)
    wb_d = nc.declare_dram_parameter("wb", [2 * CIN, 9 * COUT], BF16, isOutput=False)
    b_d = nc.declare_dram_parameter("bias", [COUT, 1], F32, isOutput=False)
    w1t_d = nc.declare_dram_parameter("w1t", [T, TR], F32, isOutput=False)
    w2t_d = nc.declare_dram_parameter("w2t", [TR, T], F32, isOutput=False)
    ones_d = nc.declare_dram_parameter("ones", [COUT, 1], F32, isOutput=False)
    onesr_d = nc.declare_dram_parameter("onesr", [1, 128], F32, isOutput=False)
    id_d = nc.declare_dram_parameter("ident", [128, 128], F32, isOutput=False)
    spk_d = nc.declare_dram_parameter("spk", [T, COUT, NPIX], FP8, isOutput=True)

    yspill_d = nc.dram_tensor("yspill", [T - R_RES, COUT, NPIX], F32)

    CG = NPIX - CD

    with ExitStack() as ctx:
        tc = ctx.enter_context(tile.TileContext(nc))

        consts = ctx.enter_context(tc.tile_pool(name="consts", bufs=1))
        xpool = ctx.enter_context(tc.tile_pool(name="xpool", bufs=2))
        respool = ctx.enter_context(tc.tile_pool(name="respool", bufs=1))
        yscr = ctx.enter_context(tc.tile_pool(name="yscr", bufs=4))
        upool = ctx.enter_context(tc.tile_pool(name="upool", bufs=2))
        vpool = ctx.enter_context(tc.tile_pool(name="vpool", bufs=1))
        spool = ctx.enter_context(tc.tile_pool(name="spool", bufs=3))
        stats = ctx.enter_context(tc.tile_pool(name="stats", bufs=1))
        psum = ctx.enter_context(tc.tile_pool(name="psum", bufs=4, space="PSUM"))

        # --- load constants/weights ---
        w_t = consts.tile([2 * CIN, 9 * COUT], F32R)
        nc.sync.dma_start(w_t[:], w_d[:])
        ewb_t = consts.tile([2 * CIN, 9 * COUT], BF16)
        nc.sync.dma_start(ewb_t[:], ewb_d[:])
        wb_t = consts.tile([2 * CIN, 9 * COUT], BF16)
        nc.sync.dma_start(wb_t[:], wb_d[:])
        b_t = consts.tile([COUT, 1], F32)
        nc.sync.dma_start(b_t[:], b_d[:])
        w1t_t = consts.tile([T, TR], F32)
        nc.sync.dma_start(w1t_t[:], w1t_d[:])
        w2t_t = consts.tile([TR, T], F32)
        nc.sync.dma_start(w2t_t[:], w2t_d[:])
        ones_t = consts.tile([COUT, 1], F32)
        nc.sync.dma_start(ones_t[:], ones_d[:])
        onesr_t = consts.tile([1, 128], F32)
        nc.sync.dma_start(onesr_t[:], onesr_d[:])
        id_t = consts.tile([128, 128], F32)
        nc.sync.dma_start(id_t[:], id_d[:])

        sums_t = stats.tile([COUT, T], F32)
        maxs_t = stats.tile([COUT, T], F32)
        nthm_t = stats.tile([COUT, 1], F32)
        nc.vector.memset(nthm_t[:], -V_TH_MINUS)

        res_y = respool.tile([COUT, R_RES * NPIX], F32)

        def conv_pair(p):
            """Emit the full-precision conv for timestep pair p (t=2p, 2p+1):
            fp32r main pass + bf16 correction passes, all accumulating into
            the same PSUM tiles. Returns the two PSUM tiles."""
            x_t = xpool.tile([2 * CIN, NPAD], F32R, tag="x", name="x")
            nc.sync.dma_start(x_t[:], x_d[p])
            xviews = [x_t[:].rearrange("p (h w) -> p h w", h=HP, w=WP)]
            lhss = [w_t]
            if WCORR:
                xb_t = xpool.tile([2 * CIN, NPAD], BF16, tag="xb", name="xb")
                nc.sync.dma_start(xb_t[:], xb_d[p])
                xviews.append(xb_t[:].rearrange("p (h w) -> p h w", h=HP, w=WP))
                lhss.append(ewb_t)
            if p in XCORR_PAIRS:
                exb_t = xpool.tile([2 * CIN, NPAD], BF16, tag="exb", name="exb")
                nc.sync.dma_start(exb_t[:], exb_d[p])
                xviews.append(exb_t[:].rearrange("p (h w) -> p h w", h=HP, w=WP))
                lhss.append(wb_t)
            npass = len(lhss)
            pys = [psum.tile([COUT, NPIX], F32, tag="py", name="py") for _ in range(2)]
            for half in range(2):
                h0 = half * 16
                outs = [
                    pys[img][:, half * 512 : (half + 1) * 512].rearrange(
                        "p (h w) -> p h w", h=16, w=W
                    )
                    for img in range(2)
                ]
                for ps in range(npass):
                    xv = xviews[ps]
                    wsrc = lhss[ps]
                    for o in range(9):
                        kh, kw = o // 3, o % 3
                        for img in range(2):
                            rhs = xv[
                                img * CIN : (img + 1) * CIN,
                                h0 + kh : h0 + kh + 16,
                                kw : kw + W,
                            ]
                            lhsT = wsrc[
                                img * CIN : (img + 1) * CIN, o * COUT : (o + 1) * COUT
                            ]
                            nc.tensor.matmul(
                                outs[img],
                                lhsT,
                                rhs,
                                start=(ps == 0 and o == 0),
                                stop=(ps == npass - 1 and o == 8),
                                tile_position=(img * CIN, 0),
                            )
            return pys

        # --- phase 1: conv all t; stats; t < R_RES resident, rest spilled ---
        for p in range(NPAIR):
            pys = conv_pair(p)
            for img in range(2):
                t = 2 * p + img
                if t < R_RES:
                    y_sb = res_y[:, t * NPIX : (t + 1) * NPIX]
                else:
                    y_sb = yscr.tile([COUT, NPIX], F32, tag="ys", name="ys")[:]
                nc.scalar.activation(
                    y_sb,
                    pys[img][:],
                    ACTF.Identity,
                    bias=b_t[:, 0:1],
                    accum_out=sums_t[:, t : t + 1],
                )
                nc.vector.tensor_reduce(
                    maxs_t[:, t : t + 1], y_sb, mybir.AxisListType.X, ALU.max
                )
                if t >= R_RES:
                    nc.sync.dma_start(yspill_d[t - R_RES], y_sb)

        # --- phase B: temporal attention (tiny) ---
        pavg_ps = psum.tile([T, 1], F32, tag="py", name="pavg_ps")
        nc.tensor.matmul(pavg_ps[:], sums_t[:], ones_t[:], start=True, stop=True)
        maxT_ps = psum.tile([T, 128], F32, tag="py", name="maxT_ps")
        nc.tensor.transpose(maxT_ps[:], maxs_t[:], id_t[:])
        pcat = stats.tile([T, 2], F32)
        nc.vector.tensor_copy(pcat[:, 0:1], pavg_ps[:])
        nc.vector.tensor_reduce(
            pcat[:, 1:2], maxT_ps[:], mybir.AxisListType.X, ALU.max
        )
        z1_ps = psum.tile([TR, 2], F32, tag="py", name="z1_ps")
        nc.tensor.matmul(z1_ps[:], w1t_t[:], pcat[:], start=True, stop=True)
        r1 = stats.tile([TR, 2], F32)
        nc.scalar.activation(r1[:], z1_ps[:], ACTF.Relu)
        z2_ps = psum.tile([1, T], F32, tag="py", name="z2_ps")
        nc.tensor.matmul(z2_ps[:], r1[:, 0:1], w2t_t[:], start=True, stop=False)
        nc.tensor.matmul(z2_ps[:], r1[:, 1:2], w2t_t[:], start=False, stop=True)
        att_row = stats.tile([1, T], F32)
        nc.scalar.activation(att_row[:], z2_ps[:], ACTF.Sigmoid)
        attB_ps = psum.tile([COUT, T], F32, tag="py", name="attB_ps")
        nc.tensor.matmul(attB_ps[:], onesr_t[:], att_row[:], start=True, stop=True)
        attB = stats.tile([COUT, T], F32)
        nc.vector.tensor_copy(attB[:], attB_ps[:])

        # --- phase 2a: prefetch spilled y back (no att dependency) ---
        scratch = {}
        for t in range(R_RES, T):
            yld = yscr.tile([COUT, NPIX], F32, tag="ys", name="ys")
            nc.sync.dma_start(yld[:], yspill_d[t - R_RES])
            scratch[t] = yld

        # --- phase 2b: IF scan over T on DVE (2 stt/step); spikes on ACT ---
        # (GpSimd/Pool is useless here: it rejects stt, its tensor_scalar
        # ucode is ~15us/op, and concurrent Pool ops destroy DVE throughput
        # via the shared SBUF ports.)
        vD = vpool.tile([COUT, NPIX], F32)
        for t in range(T):
            if t < R_RES:
                ysrc = res_y[:, t * NPIX : (t + 1) * NPIX]
            else:
                ysrc = scratch[t][:]
            att = attB[:, t : t + 1]
            uD = upool.tile([COUT, NPIX], F32, tag="uD", name="uD")
            if t == 0:
                nc.vector.tensor_scalar(uD[:], ysrc, att, None, ALU.mult)
            else:
                nc.vector.scalar_tensor_tensor(
                    uD[:], ysrc, att, vD[:], ALU.mult, ALU.add
                )
            sD = spool.tile([COUT, NPIX], FP8, tag="sD", name="sD")
            nc.scalar.activation(sD[:], uD[:], ACTF.Sign, bias=nthm_t[:, 0:1])
            nc.vector.scalar_tensor_tensor(
                vD[:], uD[:], V_TH, uD[:], ALU.is_lt, ALU.mult
            )
            nc.sync.dma_start(spk_d[t], sD[:])

    return nc


def _rhat(a):
    """Host model of the fp32r internal operand rounding."""
    a = np.asarray(a, np.float32)
    ai = a.view(np.uint32)
    if ROUND_RN:
        au = ai.astype(np.uint64)
        shift = np.uint64(23 - RBITS)
        one = np.uint64(1)
        half = one << (shift - one)
        lsb = (au >> shift) & one
        out = ((au + half - one + lsb) >> shift << shift).astype(np.uint32)
    else:
        mask = np.uint32(0xFFFFFFFF) << np.uint32(23 - RBITS)
        out = ai & mask
    return out.view(np.float32)


def prep_inputs(data, conv_w, conv_b, ta_w1, ta_w2):
    import ml_dtypes

    bf16 = ml_dtypes.bfloat16
    data = np.ascontiguousarray(np.asarray(data, dtype=np.float32))
    conv_w = np.asarray(conv_w, dtype=np.float32)
    conv_b = np.asarray(conv_b, dtype=np.float32)
    ta_w1 = np.asarray(ta_w1, dtype=np.float32)
    ta_w2 = np.asarray(ta_w2, dtype=np.float32)

    xpad = np.zeros((B, T, CIN, HP, WP), np.float32)
    xpad[:, :, :, 1 : H + 1, 1 : W + 1] = data
    xc = xpad.reshape(B, NPAIR, 2 * CIN, NPAD)
    xc_b = xc.astype(bf16)
    exc_b = (xc - _rhat(xc)).astype(bf16)

    wmat = conv_w.transpose(1, 2, 3, 0).reshape(CIN, 9, COUT)  # [ci, kh*3+kw, co]
    w_in = np.empty((2 * CIN, 9 * COUT), np.float32)
    for o in range(9):
        w_in[0:CIN, o * COUT : (o + 1) * COUT] = wmat[:, o, :]
        w_in[CIN:, o * COUT : (o + 1) * COUT] = wmat[:, o, :]
    ew_in = (w_in - _rhat(w_in)).astype(bf16)
    wb_in = w_in.astype(bf16)

    aux = {
        "w": w_in,
        "ewb": ew_in,
        "wb": wb_in,
        "bias": conv_b.reshape(COUT, 1),
        "w1t": np.ascontiguousarray(ta_w1.T),
        "w2t": np.ascontiguousarray(ta_w2.T),
        "ones": np.full((COUT, 1), 1.0 / (COUT * NPIX), np.float32),
        "onesr": np.ones((1, 128), np.float32),
        "ident": np.eye(128, dtype=np.float32),
    }
    return [
        {
            "x": np.ascontiguousarray(xc[b]),
            "xb": np.ascontiguousarray(xc_b[b]),
            "exb": np.ascontiguousarray(exc_b[b]),
            **aux,
        }
        for b in range(B)
    ]


def spikes_from_raw(raw):
    """Map the fp8 Sign-domain output {-1, 0, +1} to {0.0, 1.0} float32."""
    raw = np.asarray(raw)
    if raw.dtype == np.uint8:
        import ml_dtypes

        raw = raw.view(ml_dtypes.float8_e4m3)
    return (raw.astype(np.float32) > 0).astype(np.float32)


def kernel(data, conv_w, conv_b, ta_w1, ta_w2):
    in_maps = prep_inputs(data, conv_w, conv_b, ta_w1, ta_w2)
    nc = build_program()
    res = run_bass_kernel_spmd(nc, in_maps, list(range(N_CORES)))
    out = np.stack(
        [
            spikes_from_raw(res.results[b]["spk"]).reshape(T, COUT, H, W)
            for b in range(B)
        ],
        axis=0,
    )
    return np.ascontiguousarray(out.astype(np.float32))


# revision 11
# speedup vs baseline: 1.3820x; 1.0014x over previous
"""Trainium2 Bass kernel for nn_ConvAttLIF: Conv2d(64->128, 3x3, pad1) over
(B=8, T=60) frames -> temporal squeeze-excite attention over T -> multi-step
IF neuron (integrate, threshold 0.6, hard reset) emitting binary spikes.

Sharding: data-parallel over batch B across 8 NeuronCores (1 batch element
per core); conv weights replicated.

Conv formulation: each timestep pair is one [128, 1156] DMA with the two
zero-padded images stacked on the partition axis; the 3x3 conv is 9 shifted
K=64 matmuls per image-half accumulated in PSUM, with the two images issued
interleaved on opposite PE row strips (tile_position 0/64).

Precision scheme: the main conv matmuls run in float32r (1 cycle/row vs
fp32's 4), which rounds BOTH operands to ~11 mantissa bits (RN, measured on
HW). The resulting spike flips exceed the 2e-2 gate, so bf16 correction
matmuls accumulate the known rounding residuals into the same PSUM group:
  W-corr (all timesteps):  bf16(x) @ bf16(w - R11(w))
  X-corr (XCORR_FRAC of pairs): bf16(x - R11(x)) @ bf16(w)
Each correction pass costs 1 cycle/row, so conv = (2 + frac) cycles/row.

The IF scan is column-split across engines: DVE runs the u/v chain for cols
[0, CD), GpSimd for [CD, 1024); ACT computes spikes as Sign(u - theta') into
fp8 (values {-1, 0, +1}); host maps (raw > 0) -> {0, 1}. The spill tail of
y (t >= R_RES) round-trips DRAM in fp32, prefetched back during the scan.
"""

import sys

sys.path.insert(0, "/opt/trn_rl_repo")

import numpy as np
from contextlib import ExitStack

import concourse.bass as bass
import concourse.mybir as mybir
import concourse.tile as tile
from concourse.vector_clock import ScopedClock
from concourse.bass_utils import run_bass_kernel_spmd

B, T, CIN, H, W = 8, 60, 64, 32, 32
COUT = 128
TR = 3
HP, WP = H + 2, W + 2  # zero-padded spatial dims (34x34), padding done on host
NPAD = HP * WP  # 1156
NPIX = H * W  # 1024
V_TH = 0.6
# largest f32 strictly below V_TH; u >= VTH  <=>  u - V_TH_MINUS > 0
V_TH_MINUS = float(np.nextafter(np.float32(V_TH), np.float32(-np.inf)))
N_CORES = 8
R_RES = 32  # timesteps whose conv output stays resident in SBUF f32
RBITS = 11  # fp32r internal operand rounding: mantissa fraction bits kept
ROUND_RN = True  # True: round-to-nearest-even at RBITS; False: truncate
WCORR = True  # bf16 correction of w-side fp32r rounding (all pairs)
XCORR_FRAC = 0.4  # fraction of timestep pairs getting x-side correction
XCORR_HEAD = True  # cluster corrected pairs at the head (errors cascade forward)
# scan: DVE gets cols [0, CD) via 2 fused stt ops; GpSimd (Pool) gets cols
# [CD, NPIX) via a 4-op ts/tt decomposition (Pool rejects stt). CD=NPIX
# disables the Pool lane.
CD = 1024
F32 = mybir.dt.float32
F32R = mybir.dt.float32r
BF16 = mybir.dt.bfloat16
FP8 = mybir.dt.float8e4
ALU = mybir.AluOpType
ACTF = mybir.ActivationFunctionType

NPAIR = T // 2
# Subset of pairs receiving the X-side correction: head-clustered (early
# errors cascade through the whole scan; tail errors die quickly) or
# Bresenham-interleaved.
if XCORR_HEAD:
    XCORR_PAIRS = frozenset(range(int(round(NPAIR * XCORR_FRAC))))
else:
    XCORR_PAIRS = frozenset(
        p for p in range(NPAIR)
        if int((p + 1) * XCORR_FRAC) > int(p * XCORR_FRAC)
    )
# W-side correction: skipping tail pairs predicted fine on CPU (+25 flips)
# but broke on HW (59k flips) — root cause not tracked down; keep 0.
WCORR_DROP_TAIL = 0
WCORR_PAIRS = frozenset(range(NPAIR - WCORR_DROP_TAIL))

_drain_patched = False
_tjb_patched = False


def _legalize_single_wait(bir: bytes) -> bytes:
    """This walrus build allows at most ONE sync-wait per instruction, but the
    Tile scheduler attaches several. Hoist all but one wait of each instruction
    into single-wait EventSemaphore preludes on the same engine (same-engine
    program order preserves semantics)."""
    import orjson

    j = orjson.loads(bir)
    n = 0
    for f in j["functions"]:
        for bb in f["blocks"]:
            insts = bb.get("instructions") or []
            if not any(
                len((i.get("sync_info") or {}).get("on_wait") or []) > 1 for i in insts
            ):
                continue
            out = []
            for ins in insts:
                si = ins.get("sync_info") or {}
                waits = si.get("on_wait") or []
                if len(waits) > 1:
                    for wx in waits[:-1]:
                        n += 1
                        out.append(
                            {
                                "debug": ins.get("debug", 0),
                                "engine": ins["engine"],
                                "ins": [],
                                "name": f"wsplit-{n}",
                                "opcode": "EventSemaphore",
                                "outs": [],
                                "sync_info": {"on_update": [], "on_wait": [wx]},
                            }
                        )
                    si["on_wait"] = [waits[-1]]
                out.append(ins)
            bb["instructions"] = out
    return orjson.dumps(j)


def _patch_to_json_bytes():
    global _tjb_patched
    if _tjb_patched:
        return
    _tjb_patched = True
    orig = bass.Bass.to_json_bytes
    bass.Bass.to_json_bytes = lambda self: _legalize_single_wait(orig(self))


def _patch_tile_drain():
    """This walrus build allows only one sync-wait per CTRL instruction, but
    TileContext._drain_and_barrier puts every outstanding proc's wait on a
    single tail Drain. Split the waits across single-wait NOPs."""
    global _drain_patched
    if _drain_patched:
        return
    _drain_patched = True

    def _drain_and_barrier(self, tick_clock, wait_clock):
        gc = tick_clock.global_clock
        for proc in range(len(gc)):
            tick = gc[proc]
            if tick <= 0:
                continue
            sc = ScopedClock()
            sc.require_at_least(None, proc, tick)
            w = self.nc.sync.nop(nofuse=True)
            wait_clock.add_sem_waits(w.ins, sc)
        self.nc.sync.drain()
        self.nc.all_engine_barrier()
        popped = self.nc._tile_sem_poison_stack.pop()
        assert popped is self._sem_poison
        self.nc.clear_and_free_semaphores(list(self.sems.allocated().values()))
        self.nc.all_engine_barrier()

    tile.TileContext._drain_and_barrier = _drain_and_barrier


def build_program():
    _patch_tile_drain()
    _patch_to_json_bytes()
    nc = bass.Bass("TRN2", target_bir_lowering=False, debug=False, num_devices=N_CORES)

    # Timestep PAIRS: two padded 64-channel images stacked on the partition
    # axis -> one [128, 1156] DMA per pair; the two images run as concurrent
    # K=64 matmuls on opposite PE row strips.
    x_d = nc.declare_dram_parameter("x", [NPAIR, 2 * CIN, NPAD], F32R, isOutput=False)
    xb_d = nc.declare_dram_parameter("xb", [NPAIR, 2 * CIN, NPAD], BF16, isOutput=False)
    exb_d = nc.declare_dram_parameter(
        "exb", [NPAIR, 2 * CIN, NPAD], BF16, isOutput=False
    )
    # 9 taps as lhsT [ci, co], duplicated on both partition halves.
    w_d = nc.declare_dram_parameter("w", [2 * CIN, 9 * COUT], F32R, isOutput=False)
    ewb_d = nc.declare_dram_parameter("ewb", [2 * CIN, 9 * COUT], BF16, isOutput=False)
    wb_d = nc.declare_dram_parameter("wb", [2 * CIN, 9 * COUT], BF16, isOutput=False)
    b_d = nc.declare_dram_parameter("bias", [COUT, 1], F32, isOutput=False)
    w1t_d = nc.declare_dram_parameter("w1t", [T, TR], F32, isOutput=False)
    w2t_d = nc.declare_dram_parameter("w2t", [TR, T], F32, isOutput=False)
    ones_d = nc.declare_dram_parameter("ones", [COUT, 1], F32, isOutput=False)
    onesr_d = nc.declare_dram_parameter("onesr", [1, 128], F32, isOutput=False)
    id_d = nc.declare_dram_parameter("ident", [128, 128], F32, isOutput=False)
    spk_d = nc.declare_dram_parameter("spk", [T, COUT, NPIX], FP8, isOutput=True)

    yspill_d = nc.dram_tensor("yspill", [T - R_RES, COUT, NPIX], F32)

    CG = NPIX - CD

    with ExitStack() as ctx:
        tc = ctx.enter_context(tile.TileContext(nc))

        consts = ctx.enter_context(tc.tile_pool(name="consts", bufs=1))
        xpool = ctx.enter_context(tc.tile_pool(name="xpool", bufs=2))
        respool = ctx.enter_context(tc.tile_pool(name="respool", bufs=1))
        yscr = ctx.enter_context(tc.tile_pool(name="yscr", bufs=4))
        upool = ctx.enter_context(tc.tile_pool(name="upool", bufs=2))
        vpool = ctx.enter_context(tc.tile_pool(name="vpool", bufs=1))
        spool = ctx.enter_context(tc.tile_pool(name="spool", bufs=3))
        stats = ctx.enter_context(tc.tile_pool(name="stats", bufs=1))
        psum = ctx.enter_context(tc.tile_pool(name="psum", bufs=4, space="PSUM"))

        # --- load constants/weights ---
        w_t = consts.tile([2 * CIN, 9 * COUT], F32R)
        nc.sync.dma_start(w_t[:], w_d[:])
        ewb_t = consts.tile([2 * CIN, 9 * COUT], BF16)
        nc.sync.dma_start(ewb_t[:], ewb_d[:])
        wb_t = consts.tile([2 * CIN, 9 * COUT], BF16)
        nc.sync.dma_start(wb_t[:], wb_d[:])
        b_t = consts.tile([COUT, 1], F32)
        nc.sync.dma_start(b_t[:], b_d[:])
        w1t_t = consts.tile([T, TR], F32)
        nc.sync.dma_start(w1t_t[:], w1t_d[:])
        w2t_t = consts.tile([TR, T], F32)
        nc.sync.dma_start(w2t_t[:], w2t_d[:])
        ones_t = consts.tile([COUT, 1], F32)
        nc.sync.dma_start(ones_t[:], ones_d[:])
        onesr_t = consts.tile([1, 128], F32)
        nc.sync.dma_start(onesr_t[:], onesr_d[:])
        id_t = consts.tile([128, 128], F32)
        nc.sync.dma_start(id_t[:], id_d[:])

        sums_t = stats.tile([COUT, T], F32)
        maxs_t = stats.tile([COUT, T], F32)
        nthm_t = stats.tile([COUT, 1], F32)
        nc.vector.memset(nthm_t[:], -V_TH_MINUS)

        res_y = respool.tile([COUT, R_RES * NPIX], F32)

        def conv_pair(p):
            """Emit the full-precision conv for timestep pair p (t=2p, 2p+1):
            fp32r main pass + bf16 correction passes, all accumulating into
            the same PSUM tiles. Returns the two PSUM tiles."""
            x_t = xpool.tile([2 * CIN, NPAD], F32R, tag="x", name="x")
            nc.sync.dma_start(x_t[:], x_d[p])
            xviews = [x_t[:].rearrange("p (h w) -> p h w", h=HP, w=WP)]
            lhss = [w_t]
            if WCORR and p in WCORR_PAIRS:
                xb_t = xpool.tile([2 * CIN, NPAD], BF16, tag="xb", name="xb")
                nc.sync.dma_start(xb_t[:], xb_d[p])
                xviews.append(xb_t[:].rearrange("p (h w) -> p h w", h=HP, w=WP))
                lhss.append(ewb_t)
            if p in XCORR_PAIRS:
                exb_t = xpool.tile([2 * CIN, NPAD], BF16, tag="exb", name="exb")
                nc.sync.dma_start(exb_t[:], exb_d[p])
                xviews.append(exb_t[:].rearrange("p (h w) -> p h w", h=HP, w=WP))
                lhss.append(wb_t)
            npass = len(lhss)
            pys = [psum.tile([COUT, NPIX], F32, tag="py", name="py") for _ in range(2)]
            for half in range(2):
                h0 = half * 16
                outs = [
                    pys[img][:, half * 512 : (half + 1) * 512].rearrange(
                        "p (h w) -> p h w", h=16, w=W
                    )
                    for img in range(2)
                ]
                for ps in range(npass):
                    xv = xviews[ps]
                    wsrc = lhss[ps]
                    for o in range(9):
                        kh, kw = o // 3, o % 3
                        for img in range(2):
                            rhs = xv[
                                img * CIN : (img + 1) * CIN,
                                h0 + kh : h0 + kh + 16,
                                kw : kw + W,
                            ]
                            lhsT = wsrc[
                                img * CIN : (img + 1) * CIN, o * COUT : (o + 1) * COUT
                            ]
                            nc.tensor.matmul(
                                outs[img],
                                lhsT,
                                rhs,
                                start=(ps == 0 and o == 0),
                                stop=(ps == npass - 1 and o == 8),
                                tile_position=(img * CIN, 0),
                            )
            return pys

        # --- phase 1: conv all t; stats; t < R_RES resident, rest spilled ---
        for p in range(NPAIR):
            pys = conv_pair(p)
            for img in range(2):
                t = 2 * p + img
                if t < R_RES:
                    y_sb = res_y[:, t * NPIX : (t + 1) * NPIX]
                else:
                    y_sb = yscr.tile([COUT, NPIX], F32, tag="ys", name="ys")[:]
                nc.scalar.activation(
                    y_sb,
                    pys[img][:],
                    ACTF.Identity,
                    bias=b_t[:, 0:1],
                    accum_out=sums_t[:, t : t + 1],
                )
                nc.vector.tensor_reduce(
                    maxs_t[:, t : t + 1], y_sb, mybir.AxisListType.X, ALU.max
                )
                if t >= R_RES:
                    nc.sync.dma_start(yspill_d[t - R_RES], y_sb)

        # --- phase B: temporal attention (tiny) ---
        pavg_ps = psum.tile([T, 1], F32, tag="py", name="pavg_ps")
        nc.tensor.matmul(pavg_ps[:], sums_t[:], ones_t[:], start=True, stop=True)
        maxT_ps = psum.tile([T, 128], F32, tag="py", name="maxT_ps")
        nc.tensor.transpose(maxT_ps[:], maxs_t[:], id_t[:])
        pcat = stats.tile([T, 2], F32)
        nc.vector.tensor_copy(pcat[:, 0:1], pavg_ps[:])
        nc.vector.tensor_reduce(
            pcat[:, 1:2], maxT_ps[:], mybir.AxisListType.X, ALU.max
        )
        z1_ps = psum.tile([TR, 2], F32, tag="py", name="z1_ps")
        nc.tensor.matmul(z1_ps[:], w1t_t[:], pcat[:], start=True, stop=True)
        r1 = stats.tile([TR, 2], F32)
        nc.scalar.activation(r1[:], z1_ps[:], ACTF.Relu)
        z2_ps = psum.tile([1, T], F32, tag="py", name="z2_ps")
        nc.tensor.matmul(z2_ps[:], r1[:, 0:1], w2t_t[:], start=True, stop=False)
        nc.tensor.matmul(z2_ps[:], r1[:, 1:2], w2t_t[:], start=False, stop=True)
        att_row = stats.tile([1, T], F32)
        nc.scalar.activation(att_row[:], z2_ps[:], ACTF.Sigmoid)
        attB_ps = psum.tile([COUT, T], F32, tag="py", name="attB_ps")
        nc.tensor.matmul(attB_ps[:], onesr_t[:], att_row[:], start=True, stop=True)
        attB = stats.tile([COUT, T], F32)
        nc.vector.tensor_copy(attB[:], attB_ps[:])

        # --- phase 2a: prefetch spilled y back (no att dependency) ---
        scratch = {}
        for t in range(R_RES, T):
            yld = yscr.tile([COUT, NPIX], F32, tag="ys", name="ys")
            nc.sync.dma_start(yld[:], yspill_d[t - R_RES])
            scratch[t] = yld

        # --- phase 2b: IF scan over T on DVE (2 stt/step); spikes on ACT ---
        # (GpSimd/Pool is useless here: it rejects stt, its tensor_scalar
        # ucode is ~15us/op, and concurrent Pool ops destroy DVE throughput
        # via the shared SBUF ports.)
        vD = vpool.tile([COUT, NPIX], F32)
        for t in range(T):
            if t < R_RES:
                ysrc = res_y[:, t * NPIX : (t + 1) * NPIX]
            else:
                ysrc = scratch[t][:]
            att = attB[:, t : t + 1]
            uD = upool.tile([COUT, NPIX], F32, tag="uD", name="uD")
            if t == 0:
                nc.vector.tensor_scalar(uD[:], ysrc, att, None, ALU.mult)
            else:
                nc.vector.scalar_tensor_tensor(
                    uD[:], ysrc, att, vD[:], ALU.mult, ALU.add
                )
            sD = spool.tile([COUT, NPIX], FP8, tag="sD", name="sD")
            nc.scalar.activation(sD[:], uD[:], ACTF.Sign, bias=nthm_t[:, 0:1])
            nc.vector.scalar_tensor_tensor(
                vD[:], uD[:], V_TH, uD[:], ALU.is_lt, ALU.mult
            )
            nc.sync.dma_start(spk_d[t], sD[:])

    return nc


def _rhat(a):
    """Host model of the fp32r internal operand rounding."""
    a = np.asarray(a, np.float32)
    ai = a.view(np.uint32)
    if ROUND_RN:
        au = ai.astype(np.uint64)
        shift = np.uint64(23 - RBITS)
        one = np.uint64(1)
        half = one << (shift - one)
        lsb = (au >> shift) & one
        out = ((au + half - one + lsb) >> shift << shift).astype(np.uint32)
    else:
        mask = np.uint32(0xFFFFFFFF) << np.uint32(23 - RBITS)
        out = ai & mask
    return out.view(np.float32)


def prep_inputs(data, conv_w, conv_b, ta_w1, ta_w2):
    import ml_dtypes

    bf16 = ml_dtypes.bfloat16
    data = np.ascontiguousarray(np.asarray(data, dtype=np.float32))
    conv_w = np.asarray(conv_w, dtype=np.float32)
    conv_b = np.asarray(conv_b, dtype=np.float32)
    ta_w1 = np.asarray(ta_w1, dtype=np.float32)
    ta_w2 = np.asarray(ta_w2, dtype=np.float32)

    xpad = np.zeros((B, T, CIN, HP, WP), np.float32)
    xpad[:, :, :, 1 : H + 1, 1 : W + 1] = data
    xc = xpad.reshape(B, NPAIR, 2 * CIN, NPAD)
    xc_b = xc.astype(bf16)
    exc_b = (xc - _rhat(xc)).astype(bf16)

    wmat = conv_w.transpose(1, 2, 3, 0).reshape(CIN, 9, COUT)  # [ci, kh*3+kw, co]
    w_in = np.empty((2 * CIN, 9 * COUT), np.float32)
    for o in range(9):
        w_in[0:CIN, o * COUT : (o + 1) * COUT] = wmat[:, o, :]
        w_in[CIN:, o * COUT : (o + 1) * COUT] = wmat[:, o, :]
    ew_in = (w_in - _rhat(w_in)).astype(bf16)
    wb_in = w_in.astype(bf16)

    aux = {
        "w": w_in,
        "ewb": ew_in,
        "wb": wb_in,
        "bias": conv_b.reshape(COUT, 1),
        "w1t": np.ascontiguousarray(ta_w1.T),
        "w2t": np.ascontiguousarray(ta_w2.T),
        "ones": np.full((COUT, 1), 1.0 / (COUT * NPIX), np.float32),
        "onesr": np.ones((1, 128), np.float32),
        "ident": np.eye(128, dtype=np.float32),
    }
    return [
        {
            "x": np.ascontiguousarray(xc[b]),
            "xb": np.ascontiguousarray(xc_b[b]),
            "exb": np.ascontiguousarray(exc_b[b]),
            **aux,
        }
        for b in range(B)
    ]


def spikes_from_raw(raw):
    """Map the fp8 Sign-domain output {-1, 0, +1} to {0.0, 1.0} float32."""
    raw = np.asarray(raw)
    if raw.dtype == np.uint8:
        import ml_dtypes

        raw = raw.view(ml_dtypes.float8_e4m3)
    return (raw.astype(np.float32) > 0).astype(np.float32)


def kernel(data, conv_w, conv_b, ta_w1, ta_w2):
    in_maps = prep_inputs(data, conv_w, conv_b, ta_w1, ta_w2)
    nc = build_program()
    res = run_bass_kernel_spmd(nc, in_maps, list(range(N_CORES)))
    out = np.stack(
        [
            spikes_from_raw(res.results[b]["spk"]).reshape(T, COUT, H, W)
            for b in range(B)
        ],
        axis=0,
    )
    return np.ascontiguousarray(out.astype(np.float32))


# revision 12
# speedup vs baseline: 1.4534x; 1.0517x over previous
"""Trainium2 Bass kernel for nn_ConvAttLIF: Conv2d(64->128, 3x3, pad1) over
(B=8, T=60) frames -> temporal squeeze-excite attention over T -> multi-step
IF neuron (integrate, threshold 0.6, hard reset) emitting binary spikes.

Sharding: data-parallel over batch B across 8 NeuronCores (1 batch element
per core); conv weights replicated.

Conv formulation: each timestep pair is one [128, 1156] DMA with the two
zero-padded images stacked on the partition axis; the 3x3 conv is 9 shifted
K=64 matmuls per image-half accumulated in PSUM, with the two images issued
interleaved on opposite PE row strips (tile_position 0/64).

Precision scheme: the main conv matmuls run in float32r (1 cycle/row vs
fp32's 4), which rounds BOTH operands to ~11 mantissa bits (RN, measured on
HW). The resulting spike flips exceed the 2e-2 gate, so bf16 correction
matmuls accumulate the known rounding residuals into the same PSUM group:
  W-corr (all timesteps):  bf16(x) @ bf16(w - R11(w))
  X-corr (XCORR_FRAC of pairs): bf16(x - R11(x)) @ bf16(w)
Each correction pass costs 1 cycle/row, so conv = (2 + frac) cycles/row.

The IF scan is column-split across engines: DVE runs the u/v chain for cols
[0, CD), GpSimd for [CD, 1024); ACT computes spikes as Sign(u - theta') into
fp8 (values {-1, 0, +1}); host maps (raw > 0) -> {0, 1}. The spill tail of
y (t >= R_RES) round-trips DRAM in fp32, prefetched back during the scan.
"""

import sys

sys.path.insert(0, "/opt/trn_rl_repo")

import numpy as np
from contextlib import ExitStack

import concourse.bass as bass
import concourse.mybir as mybir
import concourse.tile as tile
from concourse.vector_clock import ScopedClock
from concourse.bass_utils import run_bass_kernel_spmd

B, T, CIN, H, W = 8, 60, 64, 32, 32
COUT = 128
TR = 3
HP, WP = H + 2, W + 2  # zero-padded spatial dims (34x34), padding done on host
NPAD = HP * WP  # 1156
NPIX = H * W  # 1024
V_TH = 0.6
# largest f32 strictly below V_TH; u >= VTH  <=>  u - V_TH_MINUS > 0
V_TH_MINUS = float(np.nextafter(np.float32(V_TH), np.float32(-np.inf)))
N_CORES = 8
R_RES = 32  # timesteps whose conv output stays resident in SBUF f32
RBITS = 11  # fp32r internal operand rounding: mantissa fraction bits kept
ROUND_RN = True  # True: round-to-nearest-even at RBITS; False: truncate
WCORR = True  # bf16 correction of w-side fp32r rounding (all pairs)
XCORR_FRAC = 0.4  # fraction of timestep pairs getting x-side correction
XCORR_HEAD = True  # cluster corrected pairs at the head (errors cascade forward)
# scan: DVE gets cols [0, CD) via 2 fused stt ops; GpSimd (Pool) gets cols
# [CD, NPIX) via a 4-op ts/tt decomposition (Pool rejects stt). CD=NPIX
# disables the Pool lane.
CD = 1024
F32 = mybir.dt.float32
F32R = mybir.dt.float32r
BF16 = mybir.dt.bfloat16
FP8 = mybir.dt.float8e4
ALU = mybir.AluOpType
ACTF = mybir.ActivationFunctionType

NPAIR = T // 2
# Subset of pairs receiving the X-side correction: head-clustered (early
# errors cascade through the whole scan; tail errors die quickly) or
# Bresenham-interleaved.
if XCORR_HEAD:
    XCORR_PAIRS = frozenset(range(int(round(NPAIR * XCORR_FRAC))))
else:
    XCORR_PAIRS = frozenset(
        p for p in range(NPAIR)
        if int((p + 1) * XCORR_FRAC) > int(p * XCORR_FRAC)
    )
# W-side correction: tail-pair residuals only affect the last few scan steps
# (CPU predictor: +25 flips for dropping the last 8 pairs).
WCORR_DROP_TAIL = 8
WCORR_PAIRS = frozenset(range(NPAIR - WCORR_DROP_TAIL))

_drain_patched = False
_tjb_patched = False


def _legalize_single_wait(bir: bytes) -> bytes:
    """This walrus build allows at most ONE sync-wait per instruction, but the
    Tile scheduler attaches several. Hoist all but one wait of each instruction
    into single-wait EventSemaphore preludes on the same engine (same-engine
    program order preserves semantics)."""
    import orjson

    j = orjson.loads(bir)
    n = 0
    for f in j["functions"]:
        for bb in f["blocks"]:
            insts = bb.get("instructions") or []
            if not any(
                len((i.get("sync_info") or {}).get("on_wait") or []) > 1 for i in insts
            ):
                continue
            out = []
            for ins in insts:
                si = ins.get("sync_info") or {}
                waits = si.get("on_wait") or []
                if len(waits) > 1:
                    for wx in waits[:-1]:
                        n += 1
                        out.append(
                            {
                                "debug": ins.get("debug", 0),
                                "engine": ins["engine"],
                                "ins": [],
                                "name": f"wsplit-{n}",
                                "opcode": "EventSemaphore",
                                "outs": [],
                                "sync_info": {"on_update": [], "on_wait": [wx]},
                            }
                        )
                    si["on_wait"] = [waits[-1]]
                out.append(ins)
            bb["instructions"] = out
    return orjson.dumps(j)


def _patch_to_json_bytes():
    global _tjb_patched
    if _tjb_patched:
        return
    _tjb_patched = True
    orig = bass.Bass.to_json_bytes
    bass.Bass.to_json_bytes = lambda self: _legalize_single_wait(orig(self))


def _patch_tile_drain():
    """This walrus build allows only one sync-wait per CTRL instruction, but
    TileContext._drain_and_barrier puts every outstanding proc's wait on a
    single tail Drain. Split the waits across single-wait NOPs."""
    global _drain_patched
    if _drain_patched:
        return
    _drain_patched = True

    def _drain_and_barrier(self, tick_clock, wait_clock):
        gc = tick_clock.global_clock
        for proc in range(len(gc)):
            tick = gc[proc]
            if tick <= 0:
                continue
            sc = ScopedClock()
            sc.require_at_least(None, proc, tick)
            w = self.nc.sync.nop(nofuse=True)
            wait_clock.add_sem_waits(w.ins, sc)
        self.nc.sync.drain()
        self.nc.all_engine_barrier()
        popped = self.nc._tile_sem_poison_stack.pop()
        assert popped is self._sem_poison
        self.nc.clear_and_free_semaphores(list(self.sems.allocated().values()))
        self.nc.all_engine_barrier()

    tile.TileContext._drain_and_barrier = _drain_and_barrier


def build_program():
    _patch_tile_drain()
    _patch_to_json_bytes()
    nc = bass.Bass("TRN2", target_bir_lowering=False, debug=False, num_devices=N_CORES)

    # Timestep PAIRS: two padded 64-channel images stacked on the partition
    # axis -> one [128, 1156] DMA per pair; the two images run as concurrent
    # K=64 matmuls on opposite PE row strips.
    x_d = nc.declare_dram_parameter("x", [NPAIR, 2 * CIN, NPAD], F32R, isOutput=False)
    xb_d = nc.declare_dram_parameter("xb", [NPAIR, 2 * CIN, NPAD], BF16, isOutput=False)
    exb_d = nc.declare_dram_parameter(
        "exb", [NPAIR, 2 * CIN, NPAD], BF16, isOutput=False
    )
    # 9 taps as lhsT [ci, co], duplicated on both partition halves.
    w_d = nc.declare_dram_parameter("w", [2 * CIN, 9 * COUT], F32R, isOutput=False)
    ewb_d = nc.declare_dram_parameter("ewb", [2 * CIN, 9 * COUT], BF16, isOutput=False)
    wb_d = nc.declare_dram_parameter("wb", [2 * CIN, 9 * COUT], BF16, isOutput=False)
    b_d = nc.declare_dram_parameter("bias", [COUT, 1], F32, isOutput=False)
    w1t_d = nc.declare_dram_parameter("w1t", [T, TR], F32, isOutput=False)
    w2t_d = nc.declare_dram_parameter("w2t", [TR, T], F32, isOutput=False)
    ones_d = nc.declare_dram_parameter("ones", [COUT, 1], F32, isOutput=False)
    onesr_d = nc.declare_dram_parameter("onesr", [1, 128], F32, isOutput=False)
    id_d = nc.declare_dram_parameter("ident", [128, 128], F32, isOutput=False)
    spk_d = nc.declare_dram_parameter("spk", [T, COUT, NPIX], FP8, isOutput=True)

    yspill_d = nc.dram_tensor("yspill", [T - R_RES, COUT, NPIX], F32)

    CG = NPIX - CD

    with ExitStack() as ctx:
        tc = ctx.enter_context(tile.TileContext(nc))

        consts = ctx.enter_context(tc.tile_pool(name="consts", bufs=1))
        xpool = ctx.enter_context(tc.tile_pool(name="xpool", bufs=2))
        respool = ctx.enter_context(tc.tile_pool(name="respool", bufs=1))
        yscr = ctx.enter_context(tc.tile_pool(name="yscr", bufs=4))
        upool = ctx.enter_context(tc.tile_pool(name="upool", bufs=2))
        vpool = ctx.enter_context(tc.tile_pool(name="vpool", bufs=1))
        spool = ctx.enter_context(tc.tile_pool(name="spool", bufs=3))
        stats = ctx.enter_context(tc.tile_pool(name="stats", bufs=1))
        psum = ctx.enter_context(tc.tile_pool(name="psum", bufs=4, space="PSUM"))

        # --- load constants/weights ---
        w_t = consts.tile([2 * CIN, 9 * COUT], F32R)
        nc.sync.dma_start(w_t[:], w_d[:])
        ewb_t = consts.tile([2 * CIN, 9 * COUT], BF16)
        nc.sync.dma_start(ewb_t[:], ewb_d[:])
        wb_t = consts.tile([2 * CIN, 9 * COUT], BF16)
        nc.sync.dma_start(wb_t[:], wb_d[:])
        b_t = consts.tile([COUT, 1], F32)
        nc.sync.dma_start(b_t[:], b_d[:])
        w1t_t = consts.tile([T, TR], F32)
        nc.sync.dma_start(w1t_t[:], w1t_d[:])
        w2t_t = consts.tile([TR, T], F32)
        nc.sync.dma_start(w2t_t[:], w2t_d[:])
        ones_t = consts.tile([COUT, 1], F32)
        nc.sync.dma_start(ones_t[:], ones_d[:])
        onesr_t = consts.tile([1, 128], F32)
        nc.sync.dma_start(onesr_t[:], onesr_d[:])
        id_t = consts.tile([128, 128], F32)
        nc.sync.dma_start(id_t[:], id_d[:])

        sums_t = stats.tile([COUT, T], F32)
        maxs_t = stats.tile([COUT, T], F32)
        nthm_t = stats.tile([COUT, 1], F32)
        nc.vector.memset(nthm_t[:], -V_TH_MINUS)

        res_y = respool.tile([COUT, R_RES * NPIX], F32)

        def conv_pair(p):
            """Emit the full-precision conv for timestep pair p (t=2p, 2p+1):
            fp32r main pass + bf16 correction passes, all accumulating into
            the same PSUM tiles. Returns the two PSUM tiles."""
            x_t = xpool.tile([2 * CIN, NPAD], F32R, tag="x", name="x")
            nc.sync.dma_start(x_t[:], x_d[p])
            xviews = [x_t[:].rearrange("p (h w) -> p h w", h=HP, w=WP)]
            lhss = [w_t]
            if WCORR and p in WCORR_PAIRS:
                xb_t = xpool.tile([2 * CIN, NPAD], BF16, tag="xb", name="xb")
                nc.sync.dma_start(xb_t[:], xb_d[p])
                xviews.append(xb_t[:].rearrange("p (h w) -> p h w", h=HP, w=WP))
                lhss.append(ewb_t)
            if p in XCORR_PAIRS:
                exb_t = xpool.tile([2 * CIN, NPAD], BF16, tag="exb", name="exb")
                nc.sync.dma_start(exb_t[:], exb_d[p])
                xviews.append(exb_t[:].rearrange("p (h w) -> p h w", h=HP, w=WP))
                lhss.append(wb_t)
            npass = len(lhss)
            pys = [psum.tile([COUT, NPIX], F32, tag="py", name="py") for _ in range(2)]
            for half in range(2):
                h0 = half * 16
                outs = [
                    pys[img][:, half * 512 : (half + 1) * 512].rearrange(
                        "p (h w) -> p h w", h=16, w=W
                    )
                    for img in range(2)
                ]
                for ps in range(npass):
                    xv = xviews[ps]
                    wsrc = lhss[ps]
                    for o in range(9):
                        kh, kw = o // 3, o % 3
                        for img in range(2):
                            rhs = xv[
                                img * CIN : (img + 1) * CIN,
                                h0 + kh : h0 + kh + 16,
                                kw : kw + W,
                            ]
                            lhsT = wsrc[
                                img * CIN : (img + 1) * CIN, o * COUT : (o + 1) * COUT
                            ]
                            nc.tensor.matmul(
                                outs[img],
                                lhsT,
                                rhs,
                                start=(ps == 0 and o == 0),
                                stop=(ps == npass - 1 and o == 8),
                                tile_position=(img * CIN, 0),
                            )
            return pys

        # --- phase 1: conv all t; stats; t < R_RES resident, rest spilled ---
        for p in range(NPAIR):
            pys = conv_pair(p)
            for img in range(2):
                t = 2 * p + img
                if t < R_RES:
                    y_sb = res_y[:, t * NPIX : (t + 1) * NPIX]
                else:
                    y_sb = yscr.tile([COUT, NPIX], F32, tag="ys", name="ys")[:]
                nc.scalar.activation(
                    y_sb,
                    pys[img][:],
                    ACTF.Identity,
                    bias=b_t[:, 0:1],
                    accum_out=sums_t[:, t : t + 1],
                )
                nc.vector.tensor_reduce(
                    maxs_t[:, t : t + 1], y_sb, mybir.AxisListType.X, ALU.max
                )
                if t >= R_RES:
                    nc.sync.dma_start(yspill_d[t - R_RES], y_sb)

        # --- phase B: temporal attention (tiny) ---
        pavg_ps = psum.tile([T, 1], F32, tag="py", name="pavg_ps")
        nc.tensor.matmul(pavg_ps[:], sums_t[:], ones_t[:], start=True, stop=True)
        maxT_ps = psum.tile([T, 128], F32, tag="py", name="maxT_ps")
        nc.tensor.transpose(maxT_ps[:], maxs_t[:], id_t[:])
        pcat = stats.tile([T, 2], F32)
        nc.vector.tensor_copy(pcat[:, 0:1], pavg_ps[:])
        nc.vector.tensor_reduce(
            pcat[:, 1:2], maxT_ps[:], mybir.AxisListType.X, ALU.max
        )
        z1_ps = psum.tile([TR, 2], F32, tag="py", name="z1_ps")
        nc.tensor.matmul(z1_ps[:], w1t_t[:], pcat[:], start=True, stop=True)
        r1 = stats.tile([TR, 2], F32)
        nc.scalar.activation(r1[:], z1_ps[:], ACTF.Relu)
        z2_ps = psum.tile([1, T], F32, tag="py", name="z2_ps")
        nc.tensor.matmul(z2_ps[:], r1[:, 0:1], w2t_t[:], start=True, stop=False)
        nc.tensor.matmul(z2_ps[:], r1[:, 1:2], w2t_t[:], start=False, stop=True)
        att_row = stats.tile([1, T], F32)
        nc.scalar.activation(att_row[:], z2_ps[:], ACTF.Sigmoid)
        attB_ps = psum.tile([COUT, T], F32, tag="py", name="attB_ps")
        nc.tensor.matmul(attB_ps[:], onesr_t[:], att_row[:], start=True, stop=True)
        attB = stats.tile([COUT, T], F32)
        nc.vector.tensor_copy(attB[:], attB_ps[:])

        # --- phase 2a: prefetch spilled y back (no att dependency) ---
        scratch = {}
        for t in range(R_RES, T):
            yld = yscr.tile([COUT, NPIX], F32, tag="ys", name="ys")
            nc.sync.dma_start(yld[:], yspill_d[t - R_RES])
            scratch[t] = yld

        # --- phase 2b: IF scan over T on DVE (2 stt/step); spikes on ACT ---
        # (GpSimd/Pool is useless here: it rejects stt, its tensor_scalar
        # ucode is ~15us/op, and concurrent Pool ops destroy DVE throughput
        # via the shared SBUF ports.)
        vD = vpool.tile([COUT, NPIX], F32)
        for t in range(T):
            if t < R_RES:
                ysrc = res_y[:, t * NPIX : (t + 1) * NPIX]
            else:
                ysrc = scratch[t][:]
            att = attB[:, t : t + 1]
            uD = upool.tile([COUT, NPIX], F32, tag="uD", name="uD")
            if t == 0:
                nc.vector.tensor_scalar(uD[:], ysrc, att, None, ALU.mult)
            else:
                nc.vector.scalar_tensor_tensor(
                    uD[:], ysrc, att, vD[:], ALU.mult, ALU.add
                )
            sD = spool.tile([COUT, NPIX], FP8, tag="sD", name="sD")
            nc.scalar.activation(sD[:], uD[:], ACTF.Sign, bias=nthm_t[:, 0:1])
            nc.vector.scalar_tensor_tensor(
                vD[:], uD[:], V_TH, uD[:], ALU.is_lt, ALU.mult
            )
            nc.sync.dma_start(spk_d[t], sD[:])

    return nc


def _rhat(a):
    """Host model of the fp32r internal operand rounding."""
    a = np.asarray(a, np.float32)
    ai = a.view(np.uint32)
    if ROUND_RN:
        au = ai.astype(np.uint64)
        shift = np.uint64(23 - RBITS)
        one = np.uint64(1)
        half = one << (shift - one)
        lsb = (au >> shift) & one
        out = ((au + half - one + lsb) >> shift << shift).astype(np.uint32)
    else:
        mask = np.uint32(0xFFFFFFFF) << np.uint32(23 - RBITS)
        out = ai & mask
    return out.view(np.float32)


def prep_inputs(data, conv_w, conv_b, ta_w1, ta_w2):
    import ml_dtypes

    bf16 = ml_dtypes.bfloat16
    data = np.ascontiguousarray(np.asarray(data, dtype=np.float32))
    conv_w = np.asarray(conv_w, dtype=np.float32)
    conv_b = np.asarray(conv_b, dtype=np.float32)
    ta_w1 = np.asarray(ta_w1, dtype=np.float32)
    ta_w2 = np.asarray(ta_w2, dtype=np.float32)

    xpad = np.zeros((B, T, CIN, HP, WP), np.float32)
    xpad[:, :, :, 1 : H + 1, 1 : W + 1] = data
    xc = xpad.reshape(B, NPAIR, 2 * CIN, NPAD)
    xc_b = xc.astype(bf16)
    exc_b = (xc - _rhat(xc)).astype(bf16)

    wmat = conv_w.transpose(1, 2, 3, 0).reshape(CIN, 9, COUT)  # [ci, kh*3+kw, co]
    w_in = np.empty((2 * CIN, 9 * COUT), np.float32)
    for o in range(9):
        w_in[0:CIN, o * COUT : (o + 1) * COUT] = wmat[:, o, :]
        w_in[CIN:, o * COUT : (o + 1) * COUT] = wmat[:, o, :]
    ew_in = (w_in - _rhat(w_in)).astype(bf16)
    wb_in = w_in.astype(bf16)

    aux = {
        "w": w_in,
        "ewb": ew_in,
        "wb": wb_in,
        "bias": conv_b.reshape(COUT, 1),
        "w1t": np.ascontiguousarray(ta_w1.T),
        "w2t": np.ascontiguousarray(ta_w2.T),
        "ones": np.full((COUT, 1), 1.0 / (COUT * NPIX), np.float32),
        "onesr": np.ones((1, 128), np.float32),
        "ident": np.eye(128, dtype=np.float32),
    }
    return [
        {
            "x": np.ascontiguousarray(xc[b]),
            "xb": np.ascontiguousarray(xc_b[b]),
            "exb": np.ascontiguousarray(exc_b[b]),
            **aux,
        }
        for b in range(B)
    ]


def spikes_from_raw(raw):
    """Map the fp8 Sign-domain output {-1, 0, +1} to {0.0, 1.0} float32."""
    raw = np.asarray(raw)
    if raw.dtype == np.uint8:
        import ml_dtypes

        raw = raw.view(ml_dtypes.float8_e4m3)
    return (raw.astype(np.float32) > 0).astype(np.float32)


def kernel(data, conv_w, conv_b, ta_w1, ta_w2):
    in_maps = prep_inputs(data, conv_w, conv_b, ta_w1, ta_w2)
    nc = build_program()
    res = run_bass_kernel_spmd(nc, in_maps, list(range(N_CORES)))
    out = np.stack(
        [
            spikes_from_raw(res.results[b]["spk"]).reshape(T, COUT, H, W)
            for b in range(B)
        ],
        axis=0,
    )
    return np.ascontiguousarray(out.astype(np.float32))
